# revision 12
# baseline (speedup 1.0000x reference)
"""Trainium2 Bass kernel for 16-head MHA (B=4, S=2048, E=1024), 8 NeuronCores.

Sharding: core c handles batch b = c//2 and head-group g = c%2 (8 heads each).
Tensor-parallel within the head group: column-parallel Wq/Wk/Wv, row-parallel
Wo; the two partial Wo outputs per batch are summed on the host.

All device matmuls run in bf16 with fp32 PSUM accumulation. Inputs are
pre-transposed on the host to feature-major layouts so every matmul contracts
over the partition dimension with no on-device transposes:
  QT/KT/VT  [E, S]   (feature, token)
  WqT/WkT/WvT [E, O] (in-feature, local out-feature), O = 512
  WoT       [O, E]   (local out-feature, out)
Output per core: OUT [E, S] fp32 = partial (Wo @ ctx^T) for this head group.
"""

import sys

sys.path.insert(0, "/opt/trn_rl_repo")

import numpy as np

# Problem constants (hardcoded; kernel.py must be self-contained).
B = 4
S = 2048
E = 1024
H = 16
D = 64
N_CORES = 8
HL = H // 2  # heads per core (head-group of 8)
O = HL * D  # 512 local output features of the q/k/v projections
IC = E // 128  # 8 contraction chunks for projections
OB = O // 128  # 4 output row-blocks (head pairs)
TB = S // 128  # 16 token blocks
KB = S // 128  # 16 key blocks per head
QCHUNK = 1024  # q columns processed per softmax tile
QC = S // QCHUNK  # 2
NV = 65  # v columns + 1 ones column for the softmax denominator

_CACHE = {}


def _build():
    import concourse.bass as bass
    import concourse.mybir as mybir
    from concourse import bacc, tile

    f32 = mybir.dt.float32
    bf16 = mybir.dt.bfloat16
    Exp = mybir.ActivationFunctionType.Exp
    Ln = mybir.ActivationFunctionType.Ln

    nc = bacc.Bacc(None, target_bir_lowering=False)

    QT = nc.dram_tensor("QT", [E, S], bf16, kind="ExternalInput")
    KT = nc.dram_tensor("KT", [E, S], bf16, kind="ExternalInput")
    VT = nc.dram_tensor("VT", [E, S], bf16, kind="ExternalInput")
    WQT = nc.dram_tensor("WQT", [E, O], bf16, kind="ExternalInput")
    WKT = nc.dram_tensor("WKT", [E, O], bf16, kind="ExternalInput")
    WVT = nc.dram_tensor("WVT", [E, O], bf16, kind="ExternalInput")
    WOT = nc.dram_tensor("WOT", [O, E], bf16, kind="ExternalInput")
    # Selector matrices for broadcasting softmax reciprocals (host-built):
    # SEL[ob].T @ recip -> [128, S] whose rows 0-63 replicate recip[2*ob]
    # and rows 64-127 replicate recip[2*ob+1].
    SEL = nc.dram_tensor("SEL", [HL, OB, 128], bf16, kind="ExternalInput")
    OUT = nc.dram_tensor("OUT", [E, S], f32, kind="ExternalOutput")

    with tile.TileContext(nc) as tc:
        with (
            tc.tile_pool(name="consts", bufs=1) as constp,
            tc.tile_pool(name="weights", bufs=1) as wp,
            tc.tile_pool(name="qkv", bufs=1) as qkvp,
        ):
            sel_sb = constp.tile([HL, OB, 128], bf16, tag="sel")
            nc.sync.dma_start(sel_sb[:], SEL[:])

            # ---- weights to SBUF ----
            wq_sb = wp.tile([128, IC, O], bf16, tag="wq")
            wk_sb = wp.tile([128, IC, O], bf16, tag="wk")
            wv_sb = wp.tile([128, IC, O], bf16, tag="wv")
            wo_sb = wp.tile([128, OB, E], bf16, tag="wo")
            for ic in range(IC):
                nc.sync.dma_start(wq_sb[:, ic, :], WQT[ic * 128 : (ic + 1) * 128, :])
                nc.sync.dma_start(wk_sb[:, ic, :], WKT[ic * 128 : (ic + 1) * 128, :])
                nc.sync.dma_start(wv_sb[:, ic, :], WVT[ic * 128 : (ic + 1) * 128, :])
            for oc in range(OB):
                nc.sync.dma_start(wo_sb[:, oc, :], WOT[oc * 128 : (oc + 1) * 128, :])

            # ---- persistent qT/kT/v in SBUF (bf16) ----
            # qT/kT tiles: [128 rows = 2 heads x 64 dims, S]
            qt_sb = [qkvp.tile([128, S], bf16, tag=f"qt{ob}", name=f"qt{ob}") for ob in range(OB)]
            kt_sb = [qkvp.tile([128, S], bf16, tag=f"kt{ob}", name=f"kt{ob}") for ob in range(OB)]
            # v tiles: [128 tokens, 8 heads x (64 v-dims + ones col)]
            v_sb = [qkvp.tile([128, HL * NV], bf16, tag=f"v{tb}", name=f"v{tb}") for tb in range(TB)]
            for tb in range(TB):
                ones_col = v_sb[tb].rearrange("p (h x) -> p h x", x=NV)[:, :, D : D + 1]
                nc.vector.memset(ones_col, 1.0)

            # ================= Phase A: projections =================
            with (
                tc.tile_pool(name="xin", bufs=12) as xinp,
                tc.tile_pool(name="psum_a", bufs=8, space="PSUM") as psa,
            ):
                # q and k projections: out rows = local feature block, cols = tokens
                for which, xdram, w3, dst in (
                    ("q", QT, wq_sb, qt_sb),
                    ("k", KT, wk_sb, kt_sb),
                ):
                    xin = [
                        xinp.tile([128, S], bf16, tag="xt", name=f"{which}in{ic}")
                        for ic in range(IC)
                    ]
                    for ic in range(IC):
                        nc.sync.dma_start(xin[ic][:], xdram[ic * 128 : (ic + 1) * 128, :])
                    for ob in range(OB):
                        for j in range(S // 512):
                            ps = psa.tile([128, 512], f32, tag="ps", name=f"ps_{which}{ob}_{j}")
                            for ic in range(IC):
                                nc.tensor.matmul(
                                    ps[:],
                                    w3[:, ic, ob * 128 : (ob + 1) * 128],
                                    xin[ic][:, j * 512 : (j + 1) * 512],
                                    start=(ic == 0),
                                    stop=(ic == IC - 1),
                                )
                            nc.scalar.copy(dst[ob][:, j * 512 : (j + 1) * 512], ps[:])

                # v projection: out rows = tokens, cols = local features
                xin = [
                    xinp.tile([128, S], bf16, tag="xt", name=f"vin{ic}")
                    for ic in range(IC)
                ]
                for ic in range(IC):
                    nc.sync.dma_start(xin[ic][:], VT[ic * 128 : (ic + 1) * 128, :])
                for tb in range(TB):
                    ps = psa.tile([128, 512], f32, tag="ps", name=f"ps_v{tb}")
                    for ic in range(IC):
                        nc.tensor.matmul(
                            ps[:],
                            xin[ic][:, tb * 128 : (tb + 1) * 128],
                            wv_sb[:, ic, :],
                            start=(ic == 0),
                            stop=(ic == IC - 1),
                        )
                    vdst = v_sb[tb].rearrange("p (h x) -> p h x", x=NV)[:, :, 0:D]
                    nc.scalar.copy(vdst, ps[:].rearrange("p (h d) -> p h d", d=D))

            # ================= Phase B: attention =================
            with tc.tile_pool(name="cun", bufs=1) as cunp:
                # unnormalized ctx^T (fp32): [128 rows = 2 heads x 64 dims, S]
                cu_sb = [cunp.tile([128, S], f32, tag=f"cu{ob}", name=f"cu{ob}") for ob in range(OB)]
                # softmax denominators: per-head [1, S] staging rows (compute
                # engines can only address base partitions 0/32/64, so rows are
                # gathered into den8 with SBUF->SBUF DMAs afterwards)
                dent = [cunp.tile([1, S], f32, tag=f"dent{h}", name=f"dent{h}") for h in range(HL)]
                den8 = cunp.tile([HL, S], f32, tag="den8")

                with (
                    tc.tile_pool(name="attn", bufs=3) as attnp,
                    tc.tile_pool(name="psum_s", bufs=2, space="PSUM") as pss,
                    tc.tile_pool(name="psum_c", bufs=2, space="PSUM") as psc,
                ):
                    for hl in range(HL):
                        ob = hl // 2
                        r0 = (hl % 2) * 64
                        for qc in range(QC):
                            q0 = qc * QCHUNK
                            pc = psc.tile([NV, QCHUNK], f32, tag="pc", name=f"pc{hl}_{qc}")
                            for kb in range(KB):
                                ps = pss.tile(
                                    [128, QCHUNK], f32, tag="ps", name=f"sc{hl}_{qc}_{kb}"
                                )
                                at = attnp.tile(
                                    [128, QCHUNK], bf16, tag="at", name=f"at{hl}_{qc}_{kb}"
                                )
                                for j in range(QCHUNK // 512):
                                    nc.tensor.matmul(
                                        ps[:, j * 512 : (j + 1) * 512],
                                        kt_sb[ob][r0 : r0 + 64, kb * 128 : (kb + 1) * 128],
                                        qt_sb[ob][r0 : r0 + 64, q0 + j * 512 : q0 + (j + 1) * 512],
                                        start=True,
                                        stop=True,
                                    )
                                # attn = exp(scores / sqrt(D)), cast to bf16
                                nc.scalar.activation(at[:], ps[:], Exp, scale=0.125)
                                for j in range(QCHUNK // 512):
                                    nc.tensor.matmul(
                                        pc[:, j * 512 : (j + 1) * 512],
                                        v_sb[kb][:, hl * NV : (hl + 1) * NV],
                                        at[:, j * 512 : (j + 1) * 512],
                                        start=(kb == 0),
                                        stop=(kb == KB - 1),
                                    )
                            nc.vector.tensor_copy(
                                cu_sb[ob][r0 : r0 + 64, q0 : q0 + QCHUNK], pc[0:D, :]
                            )
                            nc.vector.tensor_copy(
                                dent[hl][:, q0 : q0 + QCHUNK], pc[D : D + 1, :]
                            )
                    for h in range(HL):
                        nc.sync.dma_start(den8[h : h + 1, :], dent[h][:])

                # ============ Phase C: normalize ctx^T ============
                ctxt_sb = [
                    qkvp.tile([128, S], bf16, tag=f"kt{ob}", name=f"ctxt{ob}")
                    for ob in range(OB)
                ]
                with (
                    tc.tile_pool(name="norm", bufs=1) as normp,
                    tc.tile_pool(name="psum_b", bufs=2, space="PSUM") as psb,
                ):
                    # 1/den via exp(-ln(den)) on ScalarE (ACT Reciprocal is banned)
                    logd = normp.tile([HL, S], f32, tag="logd")
                    recip = normp.tile([HL, S], bf16, tag="recip")
                    nc.scalar.activation(logd[:], den8[:], Ln)
                    nc.scalar.activation(recip[:], logd[:], Exp, scale=-1.0)
                    for ob in range(OB):
                        pb = psb.tile([128, S], f32, tag="pb", name=f"pb{ob}")
                        for j in range(S // 512):
                            nc.tensor.matmul(
                                pb[:, j * 512 : (j + 1) * 512],
                                sel_sb[:, ob, :],
                                recip[:, j * 512 : (j + 1) * 512],
                                start=True,
                                stop=True,
                            )
                        nc.vector.tensor_mul(ctxt_sb[ob][:], cu_sb[ob][:], pb[:])

            # ================= Phase D: output projection =================
            with (
                tc.tile_pool(name="outs", bufs=2) as outsp,
                tc.tile_pool(name="psum_o", bufs=2, space="PSUM") as pso,
            ):
                for eb in range(E // 128):
                    po = pso.tile([128, S], f32, tag="po", name=f"po{eb}")
                    for oc in range(OB):
                        for j in range(S // 512):
                            nc.tensor.matmul(
                                po[:, j * 512 : (j + 1) * 512],
                                wo_sb[:, oc, eb * 128 : (eb + 1) * 128],
                                ctxt_sb[oc][:, j * 512 : (j + 1) * 512],
                                start=(oc == 0),
                                stop=(oc == OB - 1),
                            )
                    so = outsp.tile([128, S], f32, tag="so", name=f"so{eb}")
                    nc.scalar.copy(so[:], po[:])
                    nc.sync.dma_start(OUT[eb * 128 : (eb + 1) * 128, :], so[:])

    nc.compile()
    return nc


def _get_nc():
    if "nc" not in _CACHE:
        _CACHE["nc"] = _build()
    return _CACHE["nc"]


def _shard_inputs(Q, K, V, Wq, Wk, Wv, Wo):
    import ml_dtypes

    bf16 = ml_dtypes.bfloat16
    Q = np.asarray(Q, np.float32)
    K = np.asarray(K, np.float32)
    V = np.asarray(V, np.float32)
    Wq = np.asarray(Wq, np.float32)
    Wk = np.asarray(Wk, np.float32)
    Wv = np.asarray(Wv, np.float32)
    Wo = np.asarray(Wo, np.float32)

    sel = np.zeros((HL, OB, 128), np.float32)
    for ob in range(OB):
        sel[2 * ob, ob, 0:64] = 1.0
        sel[2 * ob + 1, ob, 64:128] = 1.0
    sel = sel.astype(bf16)

    in_maps = []
    for c in range(N_CORES):
        b, g = divmod(c, 2)
        sl = slice(g * O, (g + 1) * O)
        in_maps.append(
            {
                "SEL": sel,
                "QT": np.ascontiguousarray(Q[b].T).astype(bf16),
                "KT": np.ascontiguousarray(K[b].T).astype(bf16),
                "VT": np.ascontiguousarray(V[b].T).astype(bf16),
                "WQT": np.ascontiguousarray(Wq[sl, :].T).astype(bf16),
                "WKT": np.ascontiguousarray(Wk[sl, :].T).astype(bf16),
                "WVT": np.ascontiguousarray(Wv[sl, :].T).astype(bf16),
                "WOT": np.ascontiguousarray(Wo[:, sl].T).astype(bf16),
            }
        )
    return in_maps


def kernel(Q, K, V, mask, Wq, Wk, Wv, Wo):
    from concourse.bass_utils import run_bass_kernel_spmd

    nc = _get_nc()
    in_maps = _shard_inputs(Q, K, V, Wq, Wk, Wv, Wo)
    res = run_bass_kernel_spmd(nc, in_maps, core_ids=list(range(N_CORES)))
    out = np.empty((B, S, E), np.float32)
    for b in range(B):
        acc = res.results[2 * b]["OUT"].astype(np.float32) + res.results[
            2 * b + 1
        ]["OUT"].astype(np.float32)
        out[b] = acc.T
    return out


# revision 19
# speedup vs baseline: 503.2173x; 503.2173x over previous
"""Trainium2 Bass kernel for 16-head MHA (B=4, S=2048, E=1024), 8 NeuronCores.

Sharding: core c handles batch b = c//2 and head-group g = c%2 (8 heads each).
Tensor-parallel within the head group: column-parallel Wq/Wk/Wv, row-parallel
Wo; the two partial Wo outputs per batch are summed on the host.

All device matmuls run in bf16 with fp32 PSUM accumulation. Inputs are
pre-transposed on the host to feature-major layouts so every matmul contracts
over the partition dimension with no on-device transposes:
  QT/KT/VT  [E, S]   (feature, token)
  WqT/WkT/WvT [E, O] (in-feature, local out-feature), O = 512
  WoT       [O, E]   (local out-feature, out)
Output per core: OUT [E, S] fp32 = partial (Wo @ ctx^T) for this head group.
"""

import sys

sys.path.insert(0, "/opt/trn_rl_repo")

import numpy as np

# Problem constants (hardcoded; kernel.py must be self-contained).
B = 4
S = 2048
E = 1024
H = 16
D = 64
N_CORES = 8
HL = H // 2  # heads per core (head-group of 8)
O = HL * D  # 512 local output features of the q/k/v projections
IC = E // 128  # 8 contraction chunks for projections
OB = O // 128  # 4 output row-blocks (head pairs)
TB = S // 128  # 16 token blocks
KB = S // 128  # 16 key blocks per head
QCHUNK = 1024  # q columns processed per softmax tile
QC = S // QCHUNK  # 2
NV = 65  # v columns + 1 ones column for the softmax denominator

_CACHE = {}


def _build():
    import concourse.bass as bass
    import concourse.mybir as mybir
    from concourse import bacc, tile

    f32 = mybir.dt.float32
    bf16 = mybir.dt.bfloat16
    Exp = mybir.ActivationFunctionType.Exp
    Ln = mybir.ActivationFunctionType.Ln

    nc = bacc.Bacc(None, target_bir_lowering=False)

    QT = nc.dram_tensor("QT", [E, S], bf16, kind="ExternalInput")
    KT = nc.dram_tensor("KT", [E, S], bf16, kind="ExternalInput")
    VT = nc.dram_tensor("VT", [E, S], bf16, kind="ExternalInput")
    WQT = nc.dram_tensor("WQT", [E, O], bf16, kind="ExternalInput")
    WKT = nc.dram_tensor("WKT", [E, O], bf16, kind="ExternalInput")
    WVT = nc.dram_tensor("WVT", [E, O], bf16, kind="ExternalInput")
    WOT = nc.dram_tensor("WOT", [O, E], bf16, kind="ExternalInput")
    # Selector matrices for broadcasting softmax reciprocals (host-built):
    # SEL[ob].T @ recip -> [128, S] whose rows 0-63 replicate recip[2*ob]
    # and rows 64-127 replicate recip[2*ob+1].
    SEL = nc.dram_tensor("SEL", [HL, OB, 128], bf16, kind="ExternalInput")
    OUT = nc.dram_tensor("OUT", [E, S], f32, kind="ExternalOutput")

    with tile.TileContext(nc) as tc:
        with (
            tc.tile_pool(name="consts", bufs=1) as constp,
            tc.tile_pool(name="weights", bufs=1) as wp,
            tc.tile_pool(name="qkv", bufs=1) as qkvp,
        ):
            sel_sb = constp.tile([HL, OB, 128], bf16, tag="sel")
            nc.sync.dma_start(sel_sb[:], SEL[:])

            # ---- weights to SBUF ----
            wq_sb = wp.tile([128, IC, O], bf16, tag="wq")
            wk_sb = wp.tile([128, IC, O], bf16, tag="wk")
            wv_sb = wp.tile([128, IC, O], bf16, tag="wv")
            wo_sb = wp.tile([128, OB, E], bf16, tag="wo")
            for ic in range(IC):
                nc.sync.dma_start(wq_sb[:, ic, :], WQT[ic * 128 : (ic + 1) * 128, :])
                nc.sync.dma_start(wk_sb[:, ic, :], WKT[ic * 128 : (ic + 1) * 128, :])
                nc.sync.dma_start(wv_sb[:, ic, :], WVT[ic * 128 : (ic + 1) * 128, :])
            for oc in range(OB):
                nc.sync.dma_start(wo_sb[:, oc, :], WOT[oc * 128 : (oc + 1) * 128, :])

            # ---- persistent qT/kT/v in SBUF (bf16) ----
            # qT/kT tiles: [128 rows = 2 heads x 64 dims, S]
            qt_sb = [qkvp.tile([128, S], bf16, tag=f"qt{ob}", name=f"qt{ob}") for ob in range(OB)]
            kt_sb = [qkvp.tile([128, S], bf16, tag=f"kt{ob}", name=f"kt{ob}") for ob in range(OB)]
            # v tiles: [128 tokens, 8 heads x (64 v-dims + ones col)]
            v_sb = [qkvp.tile([128, HL * NV], bf16, tag=f"v{tb}", name=f"v{tb}") for tb in range(TB)]
            for tb in range(TB):
                ones_col = v_sb[tb].rearrange("p (h x) -> p h x", x=NV)[:, :, D : D + 1]
                nc.vector.memset(ones_col, 1.0)

            # ================= Phase A: projections =================
            with (
                tc.tile_pool(name="xin", bufs=12) as xinp,
                tc.tile_pool(name="psum_a", bufs=8, space="PSUM") as psa,
            ):
                # q and k projections: out rows = local feature block, cols = tokens
                for which, xdram, w3, dst in (
                    ("q", QT, wq_sb, qt_sb),
                    ("k", KT, wk_sb, kt_sb),
                ):
                    xin = [
                        xinp.tile([128, S], bf16, tag="xt", name=f"{which}in{ic}")
                        for ic in range(IC)
                    ]
                    for ic in range(IC):
                        nc.sync.dma_start(xin[ic][:], xdram[ic * 128 : (ic + 1) * 128, :])
                    for ob in range(OB):
                        for j in range(S // 512):
                            ps = psa.tile([128, 512], f32, tag="ps", name=f"ps_{which}{ob}_{j}")
                            for ic in range(IC):
                                nc.tensor.matmul(
                                    ps[:],
                                    w3[:, ic, ob * 128 : (ob + 1) * 128],
                                    xin[ic][:, j * 512 : (j + 1) * 512],
                                    start=(ic == 0),
                                    stop=(ic == IC - 1),
                                )
                            nc.scalar.copy(dst[ob][:, j * 512 : (j + 1) * 512], ps[:])

                # v projection: out rows = tokens, cols = local features
                xin = [
                    xinp.tile([128, S], bf16, tag="xt", name=f"vin{ic}")
                    for ic in range(IC)
                ]
                for ic in range(IC):
                    nc.sync.dma_start(xin[ic][:], VT[ic * 128 : (ic + 1) * 128, :])
                for tb in range(TB):
                    ps = psa.tile([128, 512], f32, tag="ps", name=f"ps_v{tb}")
                    for ic in range(IC):
                        nc.tensor.matmul(
                            ps[:],
                            xin[ic][:, tb * 128 : (tb + 1) * 128],
                            wv_sb[:, ic, :],
                            start=(ic == 0),
                            stop=(ic == IC - 1),
                        )
                    vdst = v_sb[tb].rearrange("p (h x) -> p h x", x=NV)[:, :, 0:D]
                    nc.scalar.copy(vdst, ps[:].rearrange("p (h d) -> p h d", d=D))

            # ================= Phase B: attention =================
            with tc.tile_pool(name="cun", bufs=1) as cunp:
                # unnormalized ctx^T (fp32): [128 rows = 2 heads x 64 dims, S]
                cu_sb = [cunp.tile([128, S], f32, tag=f"cu{ob}", name=f"cu{ob}") for ob in range(OB)]
                # softmax denominators: per-head [1, S] staging rows (compute
                # engines can only address base partitions 0/32/64, so rows are
                # gathered into den8 with SBUF->SBUF DMAs afterwards)
                dent = [cunp.tile([1, S], f32, tag=f"dent{h}", name=f"dent{h}") for h in range(HL)]
                den8 = cunp.tile([HL, S], f32, tag="den8")

                with (
                    tc.tile_pool(name="attn", bufs=3) as attnp,
                    tc.tile_pool(name="psum_s", bufs=2, space="PSUM") as pss,
                    tc.tile_pool(name="psum_c", bufs=2, space="PSUM") as psc,
                ):
                    for hl in range(HL):
                        ob = hl // 2
                        r0 = (hl % 2) * 64
                        for qc in range(QC):
                            q0 = qc * QCHUNK
                            pc = psc.tile([NV, QCHUNK], f32, tag="pc", name=f"pc{hl}_{qc}")
                            for kb in range(KB):
                                ps = pss.tile(
                                    [128, QCHUNK], f32, tag="ps", name=f"sc{hl}_{qc}_{kb}"
                                )
                                at = attnp.tile(
                                    [128, QCHUNK], bf16, tag="at", name=f"at{hl}_{qc}_{kb}"
                                )
                                for j in range(QCHUNK // 512):
                                    nc.tensor.matmul(
                                        ps[:, j * 512 : (j + 1) * 512],
                                        kt_sb[ob][r0 : r0 + 64, kb * 128 : (kb + 1) * 128],
                                        qt_sb[ob][r0 : r0 + 64, q0 + j * 512 : q0 + (j + 1) * 512],
                                        start=True,
                                        stop=True,
                                    )
                                # attn = exp(scores / sqrt(D)), cast to bf16
                                nc.scalar.activation(at[:], ps[:], Exp, scale=0.125)
                                for j in range(QCHUNK // 512):
                                    nc.tensor.matmul(
                                        pc[:, j * 512 : (j + 1) * 512],
                                        v_sb[kb][:, hl * NV : (hl + 1) * NV],
                                        at[:, j * 512 : (j + 1) * 512],
                                        start=(kb == 0),
                                        stop=(kb == KB - 1),
                                    )
                            nc.vector.tensor_copy(
                                cu_sb[ob][r0 : r0 + 64, q0 : q0 + QCHUNK], pc[0:D, :]
                            )
                            nc.vector.tensor_copy(
                                dent[hl][:, q0 : q0 + QCHUNK], pc[D : D + 1, :]
                            )
                    for h in range(HL):
                        nc.sync.dma_start(den8[h : h + 1, :], dent[h][:])

                # ============ Phase C: normalize ctx^T ============
                ctxt_sb = [
                    qkvp.tile([128, S], bf16, tag=f"kt{ob}", name=f"ctxt{ob}")
                    for ob in range(OB)
                ]
                with (
                    tc.tile_pool(name="norm", bufs=1) as normp,
                    tc.tile_pool(name="psum_b", bufs=2, space="PSUM") as psb,
                ):
                    # 1/den via exp(-ln(den)) on ScalarE (ACT Reciprocal is banned)
                    logd = normp.tile([HL, S], f32, tag="logd")
                    recip = normp.tile([HL, S], bf16, tag="recip")
                    nc.scalar.activation(logd[:], den8[:], Ln)
                    nc.scalar.activation(recip[:], logd[:], Exp, scale=-1.0)
                    for ob in range(OB):
                        pb = psb.tile([128, S], f32, tag="pb", name=f"pb{ob}")
                        for j in range(S // 512):
                            nc.tensor.matmul(
                                pb[:, j * 512 : (j + 1) * 512],
                                sel_sb[:, ob, :],
                                recip[:, j * 512 : (j + 1) * 512],
                                start=True,
                                stop=True,
                            )
                        nc.vector.tensor_mul(ctxt_sb[ob][:], cu_sb[ob][:], pb[:])

            # ================= Phase D: output projection =================
            with (
                tc.tile_pool(name="outs", bufs=2) as outsp,
                tc.tile_pool(name="psum_o", bufs=2, space="PSUM") as pso,
            ):
                for eb in range(E // 128):
                    po = pso.tile([128, S], f32, tag="po", name=f"po{eb}")
                    for oc in range(OB):
                        for j in range(S // 512):
                            nc.tensor.matmul(
                                po[:, j * 512 : (j + 1) * 512],
                                wo_sb[:, oc, eb * 128 : (eb + 1) * 128],
                                ctxt_sb[oc][:, j * 512 : (j + 1) * 512],
                                start=(oc == 0),
                                stop=(oc == OB - 1),
                            )
                    so = outsp.tile([128, S], f32, tag="so", name=f"so{eb}")
                    nc.scalar.copy(so[:], po[:])
                    nc.sync.dma_start(OUT[eb * 128 : (eb + 1) * 128, :], so[:])

    nc.compile()
    return nc


def _get_nc():
    if "nc" not in _CACHE:
        _CACHE["nc"] = _build()
    return _CACHE["nc"]


def _shard_inputs(Q, K, V, Wq, Wk, Wv, Wo):
    import ml_dtypes

    bf16 = ml_dtypes.bfloat16
    Q = np.asarray(Q, np.float32)
    K = np.asarray(K, np.float32)
    V = np.asarray(V, np.float32)
    Wq = np.asarray(Wq, np.float32)
    Wk = np.asarray(Wk, np.float32)
    Wv = np.asarray(Wv, np.float32)
    Wo = np.asarray(Wo, np.float32)

    sel = np.zeros((HL, OB, 128), np.float32)
    for ob in range(OB):
        sel[2 * ob, ob, 0:64] = 1.0
        sel[2 * ob + 1, ob, 64:128] = 1.0
    sel = sel.astype(bf16)

    in_maps = []
    for c in range(N_CORES):
        b, g = divmod(c, 2)
        sl = slice(g * O, (g + 1) * O)
        in_maps.append(
            {
                "SEL": sel,
                "QT": np.ascontiguousarray(Q[b].T).astype(bf16),
                "KT": np.ascontiguousarray(K[b].T).astype(bf16),
                "VT": np.ascontiguousarray(V[b].T).astype(bf16),
                "WQT": np.ascontiguousarray(Wq[sl, :].T).astype(bf16),
                "WKT": np.ascontiguousarray(Wk[sl, :].T).astype(bf16),
                "WVT": np.ascontiguousarray(Wv[sl, :].T).astype(bf16),
                "WOT": np.ascontiguousarray(Wo[:, sl].T).astype(bf16),
            }
        )
    return in_maps


class _Runner:
    """Compile-once executor for the SPMD bass program on 8 cores.

    Mirrors concourse.bass2jax.run_bass_via_pjrt but hoists the jit out of
    the call so repeated invocations don't re-trace/re-lower. With
    donate=False the output-shaped operands are not consumed, so calls can be
    chained (feeding outputs back in) to measure marginal device time.
    """

    def __init__(self, nc, donate=True):
        import jax
        import concourse.mybir as mybir
        from concourse import bass2jax

        bass2jax.install_neuronx_cc_hook()
        self.jax = jax
        self.nc = nc
        partition_name = (
            nc.partition_id_tensor.name if nc.partition_id_tensor else None
        )
        in_names, out_names, out_avals = [], [], []
        for alloc in nc.m.functions[0].allocations:
            if not isinstance(alloc, mybir.MemoryLocationSet):
                continue
            name = alloc.memorylocations[0].name
            if alloc.kind == "ExternalInput":
                if name != partition_name:
                    in_names.append(name)
            elif alloc.kind == "ExternalOutput":
                out_names.append(name)
                out_avals.append(
                    jax.core.ShapedArray(
                        tuple(alloc.tensor_shape), mybir.dt.np(alloc.dtype)
                    )
                )
        self.in_names = in_names
        self.out_names = out_names
        self.out_avals = out_avals
        n_params = len(in_names)
        n_outs = len(out_names)
        all_in_names = list(in_names) + list(out_names)
        if partition_name is not None:
            all_in_names.append(partition_name)
        all_in_names = tuple(all_in_names)

        def _body(*args):
            operands = list(args)
            if partition_name is not None:
                operands.append(bass2jax.partition_id_tensor())
            outs = bass2jax._bass_exec_p.bind(
                *operands,
                out_avals=tuple(out_avals),
                in_names=all_in_names,
                out_names=tuple(out_names),
                lowering_input_output_aliases=(),
                sim_require_finite=True,
                sim_require_nnan=True,
                nc=nc,
            )
            return tuple(outs)

        from concourse.bass2jax import Mesh, PartitionSpec, shard_map

        devices = jax.devices()[:N_CORES]
        mesh = Mesh(np.asarray(devices), ("core",))
        self.sharded = jax.jit(
            shard_map(
                _body,
                mesh=mesh,
                in_specs=(PartitionSpec("core"),) * (n_params + n_outs),
                out_specs=(PartitionSpec("core"),) * n_outs,
                check_rep=False,
            ),
            donate_argnums=(
                tuple(range(n_params, n_params + n_outs)) if donate else ()
            ),
            keep_unused=True,
        )

    def concat_inputs(self, in_maps):
        return [
            np.concatenate([np.asarray(m[name]) for m in in_maps], axis=0)
            for name in self.in_names
        ]

    def zero_outs(self):
        return [
            np.zeros((N_CORES * a.shape[0], *a.shape[1:]), a.dtype)
            for a in self.out_avals
        ]

    def __call__(self, concat_in, concat_zeros=None):
        if concat_zeros is None:
            concat_zeros = self.zero_outs()
        out_arrs = self.sharded(*concat_in, *concat_zeros)
        return [
            {
                name: np.asarray(out_arrs[i]).reshape(
                    N_CORES, *self.out_avals[i].shape
                )[c]
                for i, name in enumerate(self.out_names)
            }
            for c in range(N_CORES)
        ]


def _get_runner():
    if "runner" not in _CACHE:
        _CACHE["runner"] = _Runner(_get_nc())
    return _CACHE["runner"]


def kernel(Q, K, V, mask, Wq, Wk, Wv, Wo):
    runner = _get_runner()
    in_maps = _shard_inputs(Q, K, V, Wq, Wk, Wv, Wo)
    results = runner(runner.concat_inputs(in_maps))
    out = np.empty((B, S, E), np.float32)
    for b in range(B):
        acc = results[2 * b]["OUT"].astype(np.float32) + results[2 * b + 1][
            "OUT"
        ].astype(np.float32)
        out[b] = acc.T
    return out


# revision 20
# speedup vs baseline: 5710.8029x; 11.3486x over previous
"""Trainium2 Bass kernel for 16-head MHA (B=4, S=2048, E=1024), 8 NeuronCores.

Sharding: core c handles batch b = c//2 and head-group g = c%2 (8 heads each).
Tensor-parallel within the head group: column-parallel Wq/Wk/Wv, row-parallel
Wo; the two partial Wo outputs per batch are summed on the host.

All device matmuls run in bf16 with fp32 PSUM accumulation. Inputs are
pre-transposed on the host to feature-major layouts so every matmul contracts
over the partition dimension with no on-device transposes:
  QT/KT/VT  [E, S]   (feature, token)
  WqT/WkT/WvT [E, O] (in-feature, local out-feature), O = 512
  WoT       [O, E]   (local out-feature, out)
Output per core: OUT [E, S] fp32 = partial (Wo @ ctx^T) for this head group.
"""

import sys

sys.path.insert(0, "/opt/trn_rl_repo")

import numpy as np

# Problem constants (hardcoded; kernel.py must be self-contained).
B = 4
S = 2048
E = 1024
H = 16
D = 64
N_CORES = 8
HL = H // 2  # heads per core (head-group of 8)
O = HL * D  # 512 local output features of the q/k/v projections
IC = E // 128  # 8 contraction chunks for projections
OB = O // 128  # 4 output row-blocks (head pairs)
TB = S // 128  # 16 token blocks
KB = S // 128  # 16 key blocks per head
QCHUNK = 1024  # q columns processed per softmax tile
QC = S // QCHUNK  # 2
NV = 65  # v columns + 1 ones column for the softmax denominator

_CACHE = {}


def _build():
    import concourse.bass as bass
    import concourse.mybir as mybir
    from concourse import bacc, tile

    f32 = mybir.dt.float32
    bf16 = mybir.dt.bfloat16
    Exp = mybir.ActivationFunctionType.Exp
    Ln = mybir.ActivationFunctionType.Ln

    nc = bacc.Bacc(None, target_bir_lowering=False)

    QT = nc.dram_tensor("QT", [E, S], bf16, kind="ExternalInput")
    KT = nc.dram_tensor("KT", [E, S], bf16, kind="ExternalInput")
    VT = nc.dram_tensor("VT", [E, S], bf16, kind="ExternalInput")
    WQT = nc.dram_tensor("WQT", [E, O], bf16, kind="ExternalInput")
    WKT = nc.dram_tensor("WKT", [E, O], bf16, kind="ExternalInput")
    WVT = nc.dram_tensor("WVT", [E, O], bf16, kind="ExternalInput")
    WOT = nc.dram_tensor("WOT", [O, E], bf16, kind="ExternalInput")
    # Selector matrices for broadcasting softmax reciprocals (host-built):
    # SEL[ob].T @ recip -> [128, S] whose rows 0-63 replicate recip[2*ob]
    # and rows 64-127 replicate recip[2*ob+1].
    SEL = nc.dram_tensor("SEL", [HL, OB, 128], bf16, kind="ExternalInput")
    OUT = nc.dram_tensor("OUT", [E, S], f32, kind="ExternalOutput")

    with tile.TileContext(nc) as tc:
        with (
            tc.tile_pool(name="consts", bufs=1) as constp,
            tc.tile_pool(name="weights", bufs=1) as wp,
            tc.tile_pool(name="qkv", bufs=1) as qkvp,
        ):
            sel_sb = constp.tile([HL, OB, 128], bf16, tag="sel")
            nc.sync.dma_start(sel_sb[:], SEL[:])

            # ---- weights to SBUF ----
            wq_sb = wp.tile([128, IC, O], bf16, tag="wq")
            wk_sb = wp.tile([128, IC, O], bf16, tag="wk")
            wv_sb = wp.tile([128, IC, O], bf16, tag="wv")
            wo_sb = wp.tile([128, OB, E], bf16, tag="wo")
            for ic in range(IC):
                nc.sync.dma_start(wq_sb[:, ic, :], WQT[ic * 128 : (ic + 1) * 128, :])
                nc.sync.dma_start(wk_sb[:, ic, :], WKT[ic * 128 : (ic + 1) * 128, :])
                nc.sync.dma_start(wv_sb[:, ic, :], WVT[ic * 128 : (ic + 1) * 128, :])
            for oc in range(OB):
                nc.sync.dma_start(wo_sb[:, oc, :], WOT[oc * 128 : (oc + 1) * 128, :])

            # ---- persistent qT/kT/v in SBUF (bf16) ----
            # qT/kT tiles: [128 rows = 2 heads x 64 dims, S]
            qt_sb = [qkvp.tile([128, S], bf16, tag=f"qt{ob}", name=f"qt{ob}") for ob in range(OB)]
            kt_sb = [qkvp.tile([128, S], bf16, tag=f"kt{ob}", name=f"kt{ob}") for ob in range(OB)]
            # v tiles: [128 tokens, 8 heads x (64 v-dims + ones col)]
            v_sb = [qkvp.tile([128, HL * NV], bf16, tag=f"v{tb}", name=f"v{tb}") for tb in range(TB)]
            for tb in range(TB):
                ones_col = v_sb[tb].rearrange("p (h x) -> p h x", x=NV)[:, :, D : D + 1]
                nc.vector.memset(ones_col, 1.0)

            # ================= Phase A: projections =================
            with (
                tc.tile_pool(name="xin", bufs=12) as xinp,
                tc.tile_pool(name="psum_a", bufs=8, space="PSUM") as psa,
            ):
                # q and k projections: out rows = local feature block, cols = tokens
                for which, xdram, w3, dst in (
                    ("q", QT, wq_sb, qt_sb),
                    ("k", KT, wk_sb, kt_sb),
                ):
                    xin = [
                        xinp.tile([128, S], bf16, tag="xt", name=f"{which}in{ic}")
                        for ic in range(IC)
                    ]
                    for ic in range(IC):
                        nc.sync.dma_start(xin[ic][:], xdram[ic * 128 : (ic + 1) * 128, :])
                    for ob in range(OB):
                        for j in range(S // 512):
                            ps = psa.tile([128, 512], f32, tag="ps", name=f"ps_{which}{ob}_{j}")
                            for ic in range(IC):
                                nc.tensor.matmul(
                                    ps[:],
                                    w3[:, ic, ob * 128 : (ob + 1) * 128],
                                    xin[ic][:, j * 512 : (j + 1) * 512],
                                    start=(ic == 0),
                                    stop=(ic == IC - 1),
                                )
                            nc.scalar.copy(dst[ob][:, j * 512 : (j + 1) * 512], ps[:])

                # v projection: out rows = tokens, cols = local features
                xin = [
                    xinp.tile([128, S], bf16, tag="xt", name=f"vin{ic}")
                    for ic in range(IC)
                ]
                for ic in range(IC):
                    nc.sync.dma_start(xin[ic][:], VT[ic * 128 : (ic + 1) * 128, :])
                for tb in range(TB):
                    ps = psa.tile([128, 512], f32, tag="ps", name=f"ps_v{tb}")
                    for ic in range(IC):
                        nc.tensor.matmul(
                            ps[:],
                            xin[ic][:, tb * 128 : (tb + 1) * 128],
                            wv_sb[:, ic, :],
                            start=(ic == 0),
                            stop=(ic == IC - 1),
                        )
                    vdst = v_sb[tb].rearrange("p (h x) -> p h x", x=NV)[:, :, 0:D]
                    nc.scalar.copy(vdst, ps[:].rearrange("p (h d) -> p h d", d=D))

            # ================= Phase B: attention =================
            with tc.tile_pool(name="cun", bufs=1) as cunp:
                # unnormalized ctx^T (fp32): [128 rows = 2 heads x 64 dims, S]
                cu_sb = [cunp.tile([128, S], f32, tag=f"cu{ob}", name=f"cu{ob}") for ob in range(OB)]
                # softmax denominators: per-head [1, S] staging rows (compute
                # engines can only address base partitions 0/32/64, so rows are
                # gathered into den8 with SBUF->SBUF DMAs afterwards)
                dent = [cunp.tile([1, S], f32, tag=f"dent{h}", name=f"dent{h}") for h in range(HL)]
                den8 = cunp.tile([HL, S], f32, tag="den8")

                with (
                    tc.tile_pool(name="attn", bufs=3) as attnp,
                    tc.tile_pool(name="psum_s", bufs=2, space="PSUM") as pss,
                    tc.tile_pool(name="psum_c", bufs=2, space="PSUM") as psc,
                ):
                    for hl in range(HL):
                        ob = hl // 2
                        r0 = (hl % 2) * 64
                        for qc in range(QC):
                            q0 = qc * QCHUNK
                            pc = psc.tile([NV, QCHUNK], f32, tag="pc", name=f"pc{hl}_{qc}")
                            for kb in range(KB):
                                ps = pss.tile(
                                    [128, QCHUNK], f32, tag="ps", name=f"sc{hl}_{qc}_{kb}"
                                )
                                at = attnp.tile(
                                    [128, QCHUNK], bf16, tag="at", name=f"at{hl}_{qc}_{kb}"
                                )
                                for j in range(QCHUNK // 512):
                                    nc.tensor.matmul(
                                        ps[:, j * 512 : (j + 1) * 512],
                                        kt_sb[ob][r0 : r0 + 64, kb * 128 : (kb + 1) * 128],
                                        qt_sb[ob][r0 : r0 + 64, q0 + j * 512 : q0 + (j + 1) * 512],
                                        start=True,
                                        stop=True,
                                    )
                                # attn = exp(scores / sqrt(D)), cast to bf16
                                nc.scalar.activation(at[:], ps[:], Exp, scale=0.125)
                                for j in range(QCHUNK // 512):
                                    nc.tensor.matmul(
                                        pc[:, j * 512 : (j + 1) * 512],
                                        v_sb[kb][:, hl * NV : (hl + 1) * NV],
                                        at[:, j * 512 : (j + 1) * 512],
                                        start=(kb == 0),
                                        stop=(kb == KB - 1),
                                    )
                            nc.vector.tensor_copy(
                                cu_sb[ob][r0 : r0 + 64, q0 : q0 + QCHUNK], pc[0:D, :]
                            )
                            nc.vector.tensor_copy(
                                dent[hl][:, q0 : q0 + QCHUNK], pc[D : D + 1, :]
                            )
                    for h in range(HL):
                        nc.sync.dma_start(den8[h : h + 1, :], dent[h][:])

                # ============ Phase C: normalize ctx^T ============
                ctxt_sb = [
                    qkvp.tile([128, S], bf16, tag=f"kt{ob}", name=f"ctxt{ob}")
                    for ob in range(OB)
                ]
                with (
                    tc.tile_pool(name="norm", bufs=1) as normp,
                    tc.tile_pool(name="psum_b", bufs=2, space="PSUM") as psb,
                ):
                    # 1/den via exp(-ln(den)) on ScalarE (ACT Reciprocal is banned)
                    logd = normp.tile([HL, S], f32, tag="logd")
                    recip = normp.tile([HL, S], bf16, tag="recip")
                    nc.scalar.activation(logd[:], den8[:], Ln)
                    nc.scalar.activation(recip[:], logd[:], Exp, scale=-1.0)
                    for ob in range(OB):
                        pb = psb.tile([128, S], f32, tag="pb", name=f"pb{ob}")
                        for j in range(S // 512):
                            nc.tensor.matmul(
                                pb[:, j * 512 : (j + 1) * 512],
                                sel_sb[:, ob, :],
                                recip[:, j * 512 : (j + 1) * 512],
                                start=True,
                                stop=True,
                            )
                        nc.vector.tensor_mul(ctxt_sb[ob][:], cu_sb[ob][:], pb[:])

            # ================= Phase D: output projection =================
            with (
                tc.tile_pool(name="outs", bufs=2) as outsp,
                tc.tile_pool(name="psum_o", bufs=2, space="PSUM") as pso,
            ):
                for eb in range(E // 128):
                    po = pso.tile([128, S], f32, tag="po", name=f"po{eb}")
                    for oc in range(OB):
                        for j in range(S // 512):
                            nc.tensor.matmul(
                                po[:, j * 512 : (j + 1) * 512],
                                wo_sb[:, oc, eb * 128 : (eb + 1) * 128],
                                ctxt_sb[oc][:, j * 512 : (j + 1) * 512],
                                start=(oc == 0),
                                stop=(oc == OB - 1),
                            )
                    so = outsp.tile([128, S], f32, tag="so", name=f"so{eb}")
                    nc.scalar.copy(so[:], po[:])
                    nc.sync.dma_start(OUT[eb * 128 : (eb + 1) * 128, :], so[:])

    nc.compile()
    return nc


def _get_nc():
    if "nc" not in _CACHE:
        _CACHE["nc"] = _build()
    return _CACHE["nc"]


def _shard_inputs(Q, K, V, Wq, Wk, Wv, Wo):
    import ml_dtypes

    bf16 = ml_dtypes.bfloat16
    Q = np.asarray(Q, np.float32)
    K = np.asarray(K, np.float32)
    V = np.asarray(V, np.float32)
    Wq = np.asarray(Wq, np.float32)
    Wk = np.asarray(Wk, np.float32)
    Wv = np.asarray(Wv, np.float32)
    Wo = np.asarray(Wo, np.float32)

    sel = np.zeros((HL, OB, 128), np.float32)
    for ob in range(OB):
        sel[2 * ob, ob, 0:64] = 1.0
        sel[2 * ob + 1, ob, 64:128] = 1.0
    sel = sel.astype(bf16)

    in_maps = []
    for c in range(N_CORES):
        b, g = divmod(c, 2)
        sl = slice(g * O, (g + 1) * O)
        in_maps.append(
            {
                "SEL": sel,
                "QT": np.ascontiguousarray(Q[b].T).astype(bf16),
                "KT": np.ascontiguousarray(K[b].T).astype(bf16),
                "VT": np.ascontiguousarray(V[b].T).astype(bf16),
                "WQT": np.ascontiguousarray(Wq[sl, :].T).astype(bf16),
                "WKT": np.ascontiguousarray(Wk[sl, :].T).astype(bf16),
                "WVT": np.ascontiguousarray(Wv[sl, :].T).astype(bf16),
                "WOT": np.ascontiguousarray(Wo[:, sl].T).astype(bf16),
            }
        )
    return in_maps


class _Runner:
    """Compile-once executor for the SPMD bass program on 8 cores.

    Mirrors concourse.bass2jax.run_bass_via_pjrt but hoists the jit out of
    the call so repeated invocations don't re-trace/re-lower. With
    donate=False the output-shaped operands are not consumed, so calls can be
    chained (feeding outputs back in) to measure marginal device time.
    """

    def __init__(self, nc, donate=True):
        import jax
        import concourse.mybir as mybir
        from concourse import bass2jax

        bass2jax.install_neuronx_cc_hook()
        self.jax = jax
        self.nc = nc
        partition_name = (
            nc.partition_id_tensor.name if nc.partition_id_tensor else None
        )
        in_names, out_names, out_avals = [], [], []
        for alloc in nc.m.functions[0].allocations:
            if not isinstance(alloc, mybir.MemoryLocationSet):
                continue
            name = alloc.memorylocations[0].name
            if alloc.kind == "ExternalInput":
                if name != partition_name:
                    in_names.append(name)
            elif alloc.kind == "ExternalOutput":
                out_names.append(name)
                out_avals.append(
                    jax.core.ShapedArray(
                        tuple(alloc.tensor_shape), mybir.dt.np(alloc.dtype)
                    )
                )
        self.in_names = in_names
        self.out_names = out_names
        self.out_avals = out_avals
        n_params = len(in_names)
        n_outs = len(out_names)
        all_in_names = list(in_names) + list(out_names)
        if partition_name is not None:
            all_in_names.append(partition_name)
        all_in_names = tuple(all_in_names)

        def _body(*args):
            operands = list(args)
            if partition_name is not None:
                operands.append(bass2jax.partition_id_tensor())
            outs = bass2jax._bass_exec_p.bind(
                *operands,
                out_avals=tuple(out_avals),
                in_names=all_in_names,
                out_names=tuple(out_names),
                lowering_input_output_aliases=(),
                sim_require_finite=True,
                sim_require_nnan=True,
                nc=nc,
            )
            return tuple(outs)

        from concourse.bass2jax import Mesh, PartitionSpec, shard_map

        devices = jax.devices()[:N_CORES]
        mesh = Mesh(np.asarray(devices), ("core",))
        self.mesh = mesh
        self.pspec = PartitionSpec("core")
        self.sharded = jax.jit(
            shard_map(
                _body,
                mesh=mesh,
                in_specs=(PartitionSpec("core"),) * (n_params + n_outs),
                out_specs=(PartitionSpec("core"),) * n_outs,
                check_rep=False,
            ),
            donate_argnums=(
                tuple(range(n_params, n_params + n_outs)) if donate else ()
            ),
            keep_unused=True,
        )

    def concat_inputs(self, in_maps):
        return [
            np.concatenate([np.asarray(m[name]) for m in in_maps], axis=0)
            for name in self.in_names
        ]

    def zero_outs(self):
        return [
            np.zeros((N_CORES * a.shape[0], *a.shape[1:]), a.dtype)
            for a in self.out_avals
        ]

    def __call__(self, concat_in, concat_zeros=None):
        if concat_zeros is None:
            concat_zeros = self.zero_outs()
        out_arrs = self.sharded(*concat_in, *concat_zeros)
        return [
            {
                name: np.asarray(out_arrs[i]).reshape(
                    N_CORES, *self.out_avals[i].shape
                )[c]
                for i, name in enumerate(self.out_names)
            }
            for c in range(N_CORES)
        ]


def _get_runner():
    if "runner" not in _CACHE:
        _CACHE["runner"] = _Runner(_get_nc())
    return _CACHE["runner"]


def kernel(Q, K, V, mask, Wq, Wk, Wv, Wo):
    runner = _get_runner()
    in_maps = _shard_inputs(Q, K, V, Wq, Wk, Wv, Wo)
    results = runner(runner.concat_inputs(in_maps))
    out = np.empty((B, S, E), np.float32)
    for b in range(B):
        acc = results[2 * b]["OUT"].astype(np.float32) + results[2 * b + 1][
            "OUT"
        ].astype(np.float32)
        out[b] = acc.T
    return out


# revision 29
# speedup vs baseline: 5945.9995x; 1.0412x over previous
"""Trainium2 Bass kernel for 16-head MHA (B=4, S=2048, E=1024), 8 NeuronCores.

Sharding: core c handles batch b = c//2 and head-group g = c%2 (8 heads each).
Tensor-parallel within the head group: column-parallel Wq/Wk/Wv, row-parallel
Wo; the two partial Wo outputs per batch are summed on the host.

All device matmuls run in fp16 with fp32 PSUM accumulation. Inputs are
pre-transposed on the host to feature-major layouts so every matmul contracts
over the partition dimension with no on-device transposes:
  QT/KT/VT  [E, S]   (feature, token)
  WqT/WkT/WvT [E, O] (in-feature, local out-feature), O = 512
  WoT       [O, E]   (local out-feature, out)
Output per core: OUT [E, S] fp32 = partial (Wo @ ctx^T) for this head group.
"""

import sys

sys.path.insert(0, "/opt/trn_rl_repo")

import numpy as np

# Problem constants (hardcoded; kernel.py must be self-contained).
B = 4
S = 2048
E = 1024
H = 16
D = 64
N_CORES = 8
HL = H // 2  # heads per core (head-group of 8)
O = HL * D  # 512 local output features of the q/k/v projections
IC = E // 128  # 8 contraction chunks for projections
OB = O // 128  # 4 output row-blocks (head pairs)
TB = S // 128  # 16 token blocks
KB = S // 128  # 16 key blocks per head
QCHUNK = 1024  # q columns processed per softmax tile
QC = S // QCHUNK  # 2
NV = 65  # v columns + 1 ones column for the softmax denominator

_CACHE = {}


def _build(phases="ABCD"):
    import concourse.bass as bass
    import concourse.mybir as mybir
    from concourse import bacc, tile

    f32 = mybir.dt.float32
    f16 = mybir.dt.float16
    Exp = mybir.ActivationFunctionType.Exp
    Ln = mybir.ActivationFunctionType.Ln

    nc = bacc.Bacc(None, target_bir_lowering=False)

    QT = nc.dram_tensor("QT", [E, S], f16, kind="ExternalInput")
    KT = nc.dram_tensor("KT", [E, S], f16, kind="ExternalInput")
    VT = nc.dram_tensor("VT", [E, S], f16, kind="ExternalInput")
    WQT = nc.dram_tensor("WQT", [E, O], f16, kind="ExternalInput")
    WKT = nc.dram_tensor("WKT", [E, O], f16, kind="ExternalInput")
    WVT = nc.dram_tensor("WVT", [E, O], f16, kind="ExternalInput")
    WOT = nc.dram_tensor("WOT", [O, E], f16, kind="ExternalInput")
    # Selector matrices for broadcasting softmax reciprocals (host-built):
    # SEL[ob].T @ recip -> [128, S] whose rows 0-63 replicate recip[2*ob]
    # and rows 64-127 replicate recip[2*ob+1].
    SEL = nc.dram_tensor("SEL", [HL, OB, 128], f16, kind="ExternalInput")
    OUT = nc.dram_tensor("OUT", [E, S], f16, kind="ExternalOutput")

    with tile.TileContext(nc) as tc:
        with (
            tc.tile_pool(name="consts", bufs=1) as constp,
            tc.tile_pool(name="weights", bufs=1) as wp,
            tc.tile_pool(name="qkv", bufs=1) as qkvp,
        ):
            sel_sb = constp.tile([HL, OB, 128], f16, tag="sel")
            nc.scalar.dma_start(sel_sb[:], SEL[:])

            # ---- weights to SBUF ----
            wq_sb = wp.tile([128, IC, O], f16, tag="wq")
            wk_sb = wp.tile([128, IC, O], f16, tag="wk")
            wv_sb = wp.tile([128, IC, O], f16, tag="wv")
            wo_sb = wp.tile([128, OB, E], f16, tag="wo")
            for ic in range(IC):
                nc.sync.dma_start(wq_sb[:, ic, :], WQT[ic * 128 : (ic + 1) * 128, :])
                nc.scalar.dma_start(wk_sb[:, ic, :], WKT[ic * 128 : (ic + 1) * 128, :])
                nc.gpsimd.dma_start(wv_sb[:, ic, :], WVT[ic * 128 : (ic + 1) * 128, :])
            for oc in range(OB):
                nc.sync.dma_start(wo_sb[:, oc, :], WOT[oc * 128 : (oc + 1) * 128, :])

            # ---- persistent qT/kT/v in SBUF (f16) ----
            # qT/kT tiles: [128 rows = 2 heads x 64 dims, S]
            qt_sb = [qkvp.tile([128, S], f16, tag=f"qt{ob}", name=f"qt{ob}") for ob in range(OB)]
            kt_sb = [qkvp.tile([128, S], f16, tag=f"kt{ob}", name=f"kt{ob}") for ob in range(OB)]
            # v tiles: [128 tokens, 8 heads x (64 v-dims + ones col)]
            v_sb = [qkvp.tile([128, HL * NV], f16, tag=f"v{tb}", name=f"v{tb}") for tb in range(TB)]
            for tb in range(TB):
                ones_col = v_sb[tb].rearrange("p (h x) -> p h x", x=NV)[:, :, D : D + 1]
                nc.vector.memset(ones_col, 1.0)

            # ================= Phase A: projections =================
            with (
                tc.tile_pool(name="xin", bufs=12) as xinp,
                tc.tile_pool(name="psum_a", bufs=8, space="PSUM") as psa,
            ):
                # q and k projections: out rows = local feature block, cols = tokens
                for which, xdram, w3, dst, dmae in (
                    ("q", QT, wq_sb, qt_sb, nc.sync),
                    ("k", KT, wk_sb, kt_sb, nc.scalar),
                ):
                    xin = [
                        xinp.tile([128, S], f16, tag="xt", name=f"{which}in{ic}")
                        for ic in range(IC)
                    ]
                    for ic in range(IC):
                        dmae.dma_start(xin[ic][:], xdram[ic * 128 : (ic + 1) * 128, :])
                    for ob in range(OB):
                        for j in range(S // 512):
                            ps = psa.tile([128, 512], f32, tag="ps", name=f"ps_{which}{ob}_{j}")
                            for ic in range(IC):
                                nc.tensor.matmul(
                                    ps[:],
                                    w3[:, ic, ob * 128 : (ob + 1) * 128],
                                    xin[ic][:, j * 512 : (j + 1) * 512],
                                    start=(ic == 0),
                                    stop=(ic == IC - 1),
                                )
                            nc.scalar.copy(dst[ob][:, j * 512 : (j + 1) * 512], ps[:])

                # v projection: out rows = tokens, cols = local features
                xin = [
                    xinp.tile([128, S], f16, tag="xt", name=f"vin{ic}")
                    for ic in range(IC)
                ]
                for ic in range(IC):
                    nc.gpsimd.dma_start(xin[ic][:], VT[ic * 128 : (ic + 1) * 128, :])
                for tb in range(TB):
                    ps = psa.tile([128, 512], f32, tag="ps", name=f"ps_v{tb}")
                    for ic in range(IC):
                        nc.tensor.matmul(
                            ps[:],
                            xin[ic][:, tb * 128 : (tb + 1) * 128],
                            wv_sb[:, ic, :],
                            start=(ic == 0),
                            stop=(ic == IC - 1),
                        )
                    vdst = v_sb[tb].rearrange("p (h x) -> p h x", x=NV)[:, :, 0:D]
                    nc.scalar.copy(vdst, ps[:].rearrange("p (h d) -> p h d", d=D))

            # ================= Phase B: attention =================
            with tc.tile_pool(name="cun", bufs=1) as cunp:
                # unnormalized ctx^T (fp32): [128 rows = 2 heads x 64 dims, S]
                cu_sb = [cunp.tile([128, S], f32, tag=f"cu{ob}", name=f"cu{ob}") for ob in range(OB)]
                # softmax denominators: per-head [1, S] staging rows (compute
                # engines can only address base partitions 0/32/64, so rows are
                # gathered into den8 with SBUF->SBUF DMAs afterwards)
                dent = [cunp.tile([1, S], f32, tag=f"dent{h}", name=f"dent{h}") for h in range(HL)]
                den8 = cunp.tile([HL, S], f32, tag="den8")

                with (
                    tc.tile_pool(name="attn", bufs=3) as attnp,
                    tc.tile_pool(name="psum_s", bufs=2, space="PSUM") as pss,
                    tc.tile_pool(name="psum_c", bufs=2, space="PSUM") as psc,
                ):
                    for hl in range(HL if "B" in phases else 0):
                        ob = hl // 2
                        r0 = (hl % 2) * 64
                        for qc in range(QC):
                            q0 = qc * QCHUNK
                            pc = psc.tile([NV, QCHUNK], f32, tag="pc", name=f"pc{hl}_{qc}")
                            for kb in range(KB):
                                ps = pss.tile(
                                    [128, QCHUNK], f32, tag="ps", name=f"sc{hl}_{qc}_{kb}"
                                )
                                at = attnp.tile(
                                    [128, QCHUNK], f16, tag="at", name=f"at{hl}_{qc}_{kb}"
                                )
                                for j in range(QCHUNK // 512):
                                    nc.tensor.matmul(
                                        ps[:, j * 512 : (j + 1) * 512],
                                        kt_sb[ob][r0 : r0 + 64, kb * 128 : (kb + 1) * 128],
                                        qt_sb[ob][r0 : r0 + 64, q0 + j * 512 : q0 + (j + 1) * 512],
                                        start=True,
                                        stop=True,
                                    )
                                # attn = exp(scores / sqrt(D)), cast to bf16
                                nc.scalar.activation(at[:], ps[:], Exp, scale=0.125)
                                for j in range(QCHUNK // 512):
                                    nc.tensor.matmul(
                                        pc[:, j * 512 : (j + 1) * 512],
                                        v_sb[kb][:, hl * NV : (hl + 1) * NV],
                                        at[:, j * 512 : (j + 1) * 512],
                                        start=(kb == 0),
                                        stop=(kb == KB - 1),
                                    )
                            nc.vector.tensor_copy(
                                cu_sb[ob][r0 : r0 + 64, q0 : q0 + QCHUNK], pc[0:D, :]
                            )
                            nc.vector.tensor_copy(
                                dent[hl][:, q0 : q0 + QCHUNK], pc[D : D + 1, :]
                            )
                    for h in range(HL if "B" in phases else 0):
                        nc.sync.dma_start(den8[h : h + 1, :], dent[h][:])

                # ============ Phase C: normalize ctx^T ============
                ctxt_sb = [
                    qkvp.tile([128, S], f16, tag=f"kt{ob}", name=f"ctxt{ob}")
                    for ob in range(OB)
                ]
                with (
                    tc.tile_pool(name="norm", bufs=1) as normp,
                    tc.tile_pool(name="psum_b", bufs=2, space="PSUM") as psb,
                ):
                    # 1/den via exp(-ln(den)) on ScalarE (ACT Reciprocal is banned)
                    logd = normp.tile([HL, S], f32, tag="logd")
                    recip = normp.tile([HL, S], f16, tag="recip")
                    if "C" in phases:
                        nc.scalar.activation(logd[:], den8[:], Ln)
                        nc.scalar.activation(recip[:], logd[:], Exp, scale=-1.0)
                    for ob in range(OB if "C" in phases else 0):
                        pb = psb.tile([128, S], f32, tag="pb", name=f"pb{ob}")
                        for j in range(S // 512):
                            nc.tensor.matmul(
                                pb[:, j * 512 : (j + 1) * 512],
                                sel_sb[:, ob, :],
                                recip[:, j * 512 : (j + 1) * 512],
                                start=True,
                                stop=True,
                            )
                        nc.vector.tensor_mul(ctxt_sb[ob][:], cu_sb[ob][:], pb[:])

            # ================= Phase D: output projection =================
            with (
                tc.tile_pool(name="outs", bufs=2) as outsp,
                tc.tile_pool(name="psum_o", bufs=2, space="PSUM") as pso,
            ):
                for eb in range(E // 128 if "D" in phases else 0):
                    po = pso.tile([128, S], f32, tag="po", name=f"po{eb}")
                    for oc in range(OB):
                        for j in range(S // 512):
                            nc.tensor.matmul(
                                po[:, j * 512 : (j + 1) * 512],
                                wo_sb[:, oc, eb * 128 : (eb + 1) * 128],
                                ctxt_sb[oc][:, j * 512 : (j + 1) * 512],
                                start=(oc == 0),
                                stop=(oc == OB - 1),
                            )
                    so = outsp.tile([128, S], f16, tag="so", name=f"so{eb}")
                    nc.scalar.copy(so[:], po[:])
                    [nc.sync, nc.scalar, nc.gpsimd][eb % 3].dma_start(
                        OUT[eb * 128 : (eb + 1) * 128, :], so[:]
                    )

    nc.compile()
    return nc


def _get_nc():
    if "nc" not in _CACHE:
        _CACHE["nc"] = _build()
    return _CACHE["nc"]


def _shard_inputs(Q, K, V, Wq, Wk, Wv, Wo):
    f16 = np.float16
    Q = np.asarray(Q, np.float32)
    K = np.asarray(K, np.float32)
    V = np.asarray(V, np.float32)
    Wq = np.asarray(Wq, np.float32)
    Wk = np.asarray(Wk, np.float32)
    Wv = np.asarray(Wv, np.float32)
    Wo = np.asarray(Wo, np.float32)

    sel = np.zeros((HL, OB, 128), np.float32)
    for ob in range(OB):
        sel[2 * ob, ob, 0:64] = 1.0
        sel[2 * ob + 1, ob, 64:128] = 1.0
    sel = sel.astype(f16)

    in_maps = []
    for c in range(N_CORES):
        b, g = divmod(c, 2)
        sl = slice(g * O, (g + 1) * O)
        in_maps.append(
            {
                "SEL": sel,
                "QT": np.ascontiguousarray(Q[b].T).astype(f16),
                "KT": np.ascontiguousarray(K[b].T).astype(f16),
                "VT": np.ascontiguousarray(V[b].T).astype(f16),
                "WQT": np.ascontiguousarray(Wq[sl, :].T).astype(f16),
                "WKT": np.ascontiguousarray(Wk[sl, :].T).astype(f16),
                "WVT": np.ascontiguousarray(Wv[sl, :].T).astype(f16),
                "WOT": np.ascontiguousarray(Wo[:, sl].T).astype(f16),
            }
        )
    return in_maps


class _Runner:
    """Compile-once executor for the SPMD bass program on 8 cores.

    Mirrors concourse.bass2jax.run_bass_via_pjrt but hoists the jit out of
    the call so repeated invocations don't re-trace/re-lower. With
    donate=False the output-shaped operands are not consumed, so calls can be
    chained (feeding outputs back in) to measure marginal device time.
    """

    def __init__(self, nc, donate=True):
        import jax
        import concourse.mybir as mybir
        from concourse import bass2jax

        bass2jax.install_neuronx_cc_hook()
        self.jax = jax
        self.nc = nc
        partition_name = (
            nc.partition_id_tensor.name if nc.partition_id_tensor else None
        )
        in_names, out_names, out_avals = [], [], []
        for alloc in nc.m.functions[0].allocations:
            if not isinstance(alloc, mybir.MemoryLocationSet):
                continue
            name = alloc.memorylocations[0].name
            if alloc.kind == "ExternalInput":
                if name != partition_name:
                    in_names.append(name)
            elif alloc.kind == "ExternalOutput":
                out_names.append(name)
                out_avals.append(
                    jax.core.ShapedArray(
                        tuple(alloc.tensor_shape), mybir.dt.np(alloc.dtype)
                    )
                )
        self.in_names = in_names
        self.out_names = out_names
        self.out_avals = out_avals
        n_params = len(in_names)
        n_outs = len(out_names)
        all_in_names = list(in_names) + list(out_names)
        if partition_name is not None:
            all_in_names.append(partition_name)
        all_in_names = tuple(all_in_names)

        def _body(*args):
            operands = list(args)
            if partition_name is not None:
                operands.append(bass2jax.partition_id_tensor())
            outs = bass2jax._bass_exec_p.bind(
                *operands,
                out_avals=tuple(out_avals),
                in_names=all_in_names,
                out_names=tuple(out_names),
                lowering_input_output_aliases=(),
                sim_require_finite=True,
                sim_require_nnan=True,
                nc=nc,
            )
            return tuple(outs)

        from concourse.bass2jax import Mesh, PartitionSpec, shard_map

        devices = jax.devices()[:N_CORES]
        mesh = Mesh(np.asarray(devices), ("core",))
        self.mesh = mesh
        self.pspec = PartitionSpec("core")
        self.sharded = jax.jit(
            shard_map(
                _body,
                mesh=mesh,
                in_specs=(PartitionSpec("core"),) * (n_params + n_outs),
                out_specs=(PartitionSpec("core"),) * n_outs,
                check_rep=False,
            ),
            donate_argnums=(
                tuple(range(n_params, n_params + n_outs)) if donate else ()
            ),
            keep_unused=True,
        )

    def concat_inputs(self, in_maps):
        return [
            np.concatenate([np.asarray(m[name]) for m in in_maps], axis=0)
            for name in self.in_names
        ]

    def zero_outs(self):
        return [
            np.zeros((N_CORES * a.shape[0], *a.shape[1:]), a.dtype)
            for a in self.out_avals
        ]

    def __call__(self, concat_in, concat_zeros=None):
        if concat_zeros is None:
            concat_zeros = self.zero_outs()
        out_arrs = self.sharded(*concat_in, *concat_zeros)
        return [
            {
                name: np.asarray(out_arrs[i]).reshape(
                    N_CORES, *self.out_avals[i].shape
                )[c]
                for i, name in enumerate(self.out_names)
            }
            for c in range(N_CORES)
        ]


def _get_runner():
    if "runner" not in _CACHE:
        _CACHE["runner"] = _Runner(_get_nc())
    return _CACHE["runner"]


def kernel(Q, K, V, mask, Wq, Wk, Wv, Wo):
    runner = _get_runner()
    in_maps = _shard_inputs(Q, K, V, Wq, Wk, Wv, Wo)
    results = runner(runner.concat_inputs(in_maps))
    out = np.empty((B, S, E), np.float32)
    for b in range(B):
        acc = results[2 * b]["OUT"].astype(np.float32) + results[2 * b + 1][
            "OUT"
        ].astype(np.float32)
        out[b] = acc.T
    return out


# revision 32
# speedup vs baseline: 6083.9649x; 1.0232x over previous
"""Trainium2 Bass kernel for 16-head MHA (B=4, S=2048, E=1024), 8 NeuronCores.

Sharding: core c handles batch b = c//2 and head-group g = c%2 (8 heads each).
Tensor-parallel within the head group: column-parallel Wq/Wk/Wv, row-parallel
Wo; the two partial Wo outputs per batch are summed on the host.

All device matmuls run in fp16 with fp32 PSUM accumulation. Inputs are
pre-transposed on the host to feature-major layouts so every matmul contracts
over the partition dimension with no on-device transposes:
  QT/KT/VT  [E, S]   (feature, token)
  WqT/WkT/WvT [E, O] (in-feature, local out-feature), O = 512
  WoT       [O, E]   (local out-feature, out)
Output per core: OUT [E, S] fp32 = partial (Wo @ ctx^T) for this head group.
"""

import sys

sys.path.insert(0, "/opt/trn_rl_repo")

import numpy as np

# Problem constants (hardcoded; kernel.py must be self-contained).
B = 4
S = 2048
E = 1024
H = 16
D = 64
N_CORES = 8
HL = H // 2  # heads per core (head-group of 8)
O = HL * D  # 512 local output features of the q/k/v projections
IC = E // 128  # 8 contraction chunks for projections
OB = O // 128  # 4 output row-blocks (head pairs)
TB = S // 128  # 16 token blocks
KB = S // 128  # 16 key blocks per head
QCHUNK = 1024  # q columns processed per softmax tile
QC = S // QCHUNK  # 2
NV = 65  # v columns + 1 ones column for the softmax denominator

_CACHE = {}


def _build(phases="ABCD"):
    import concourse.bass as bass
    import concourse.mybir as mybir
    from concourse import bacc, tile

    f32 = mybir.dt.float32
    f16 = mybir.dt.float16
    Exp = mybir.ActivationFunctionType.Exp
    Ln = mybir.ActivationFunctionType.Ln

    nc = bacc.Bacc(None, target_bir_lowering=False)

    QT = nc.dram_tensor("QT", [E, S], f16, kind="ExternalInput")
    KT = nc.dram_tensor("KT", [E, S], f16, kind="ExternalInput")
    VT = nc.dram_tensor("VT", [E, S], f16, kind="ExternalInput")
    WQT = nc.dram_tensor("WQT", [E, O], f16, kind="ExternalInput")
    WKT = nc.dram_tensor("WKT", [E, O], f16, kind="ExternalInput")
    WVT = nc.dram_tensor("WVT", [E, O], f16, kind="ExternalInput")
    WOT = nc.dram_tensor("WOT", [O, E], f16, kind="ExternalInput")
    # Selector matrices for broadcasting softmax reciprocals (host-built):
    # SEL[ob].T @ recip -> [128, S] whose rows 0-63 replicate recip[2*ob]
    # and rows 64-127 replicate recip[2*ob+1].
    SEL = nc.dram_tensor("SEL", [HL, OB, 128], f16, kind="ExternalInput")
    OUT = nc.dram_tensor("OUT", [E, S], f16, kind="ExternalOutput")

    with tile.TileContext(nc) as tc:
        with (
            tc.tile_pool(name="consts", bufs=1) as constp,
            tc.tile_pool(name="weights", bufs=1) as wp,
            tc.tile_pool(name="qkv", bufs=1) as qkvp,
        ):
            sel_sb = constp.tile([HL, OB, 128], f16, tag="sel")
            nc.scalar.dma_start(sel_sb[:], SEL[:])

            # ---- weights to SBUF ----
            wq_sb = wp.tile([128, IC, O], f16, tag="wq")
            wk_sb = wp.tile([128, IC, O], f16, tag="wk")
            wv_sb = wp.tile([128, IC, O], f16, tag="wv")
            wo_sb = wp.tile([128, OB, E], f16, tag="wo")
            for ic in range(IC):
                nc.sync.dma_start(wq_sb[:, ic, :], WQT[ic * 128 : (ic + 1) * 128, :])
                nc.scalar.dma_start(wk_sb[:, ic, :], WKT[ic * 128 : (ic + 1) * 128, :])
                nc.gpsimd.dma_start(wv_sb[:, ic, :], WVT[ic * 128 : (ic + 1) * 128, :])
            for oc in range(OB):
                nc.sync.dma_start(wo_sb[:, oc, :], WOT[oc * 128 : (oc + 1) * 128, :])

            # ---- persistent qT/kT/v in SBUF (f16) ----
            # qT/kT tiles: [128 rows = 2 heads x 64 dims, S]
            qt_sb = [qkvp.tile([128, S], f16, tag=f"qt{ob}", name=f"qt{ob}") for ob in range(OB)]
            kt_sb = [qkvp.tile([128, S], f16, tag=f"kt{ob}", name=f"kt{ob}") for ob in range(OB)]
            # v tiles: [128 tokens, 8 heads x (64 v-dims + ones col)]
            v_sb = [qkvp.tile([128, HL * NV], f16, tag=f"v{tb}", name=f"v{tb}") for tb in range(TB)]
            for tb in range(TB):
                ones_col = v_sb[tb].rearrange("p (h x) -> p h x", x=NV)[:, :, D : D + 1]
                nc.vector.memset(ones_col, 1.0)

            # ================= Phase A: projections =================
            with (
                tc.tile_pool(name="xin", bufs=12) as xinp,
                tc.tile_pool(name="psum_a", bufs=8, space="PSUM") as psa,
            ):
                # q and k projections: out rows = local feature block, cols = tokens
                for which, xdram, w3, dst, dmae in (
                    ("q", QT, wq_sb, qt_sb, nc.sync),
                    ("k", KT, wk_sb, kt_sb, nc.scalar),
                ):
                    xin = [
                        xinp.tile([128, S], f16, tag="xt", name=f"{which}in{ic}")
                        for ic in range(IC)
                    ]
                    for ic in range(IC):
                        if "Z" in phases:
                            nc.gpsimd.memset(xin[ic][:], 0.5)
                        else:
                            dmae.dma_start(xin[ic][:], xdram[ic * 128 : (ic + 1) * 128, :])
                    for ob in range(OB if "Y" not in phases else 0):
                        # ic outer so each stationary weight block is loaded
                        # once and reused for all four moving chunks
                        pss4 = [
                            psa.tile([128, 512], f32, tag="ps", name=f"ps_{which}{ob}_{j}")
                            for j in range(S // 512)
                        ]
                        for ic in range(IC):
                            for j in range(S // 512):
                                nc.tensor.matmul(
                                    pss4[j][:],
                                    w3[:, ic, ob * 128 : (ob + 1) * 128],
                                    xin[ic][:, j * 512 : (j + 1) * 512],
                                    start=(ic == 0),
                                    stop=(ic == IC - 1),
                                )
                        for j in range(S // 512):
                            nc.vector.tensor_copy(
                                dst[ob][:, j * 512 : (j + 1) * 512], pss4[j][:]
                            )

                # v projection: out rows = tokens, cols = local features
                xin = [
                    xinp.tile([128, S], f16, tag="xt", name=f"vin{ic}")
                    for ic in range(IC)
                ]
                for ic in range(IC):
                    if "Z" in phases:
                        nc.gpsimd.memset(xin[ic][:], 0.5)
                    else:
                        nc.gpsimd.dma_start(xin[ic][:], VT[ic * 128 : (ic + 1) * 128, :])
                for tb in range(TB if "Y" not in phases else 0):
                    ps = psa.tile([128, 512], f32, tag="ps", name=f"ps_v{tb}")
                    for ic in range(IC):
                        nc.tensor.matmul(
                            ps[:],
                            xin[ic][:, tb * 128 : (tb + 1) * 128],
                            wv_sb[:, ic, :],
                            start=(ic == 0),
                            stop=(ic == IC - 1),
                        )
                    vdst = v_sb[tb].rearrange("p (h x) -> p h x", x=NV)[:, :, 0:D]
                    nc.scalar.copy(vdst, ps[:].rearrange("p (h d) -> p h d", d=D))

            # ================= Phase B: attention =================
            with tc.tile_pool(name="cun", bufs=1) as cunp:
                # unnormalized ctx^T (fp32): [128 rows = 2 heads x 64 dims, S]
                cu_sb = [cunp.tile([128, S], f32, tag=f"cu{ob}", name=f"cu{ob}") for ob in range(OB)]
                # softmax denominators: per-head [1, S] staging rows (compute
                # engines can only address base partitions 0/32/64, so rows are
                # gathered into den8 with SBUF->SBUF DMAs afterwards)
                dent = [cunp.tile([1, S], f32, tag=f"dent{h}", name=f"dent{h}") for h in range(HL)]
                den8 = cunp.tile([HL, S], f32, tag="den8")

                with (
                    tc.tile_pool(name="attn", bufs=3) as attnp,
                    tc.tile_pool(name="psum_s", bufs=2, space="PSUM") as pss,
                    tc.tile_pool(name="psum_c", bufs=2, space="PSUM") as psc,
                ):
                    for hl in range(HL if "B" in phases else 0):
                        ob = hl // 2
                        r0 = (hl % 2) * 64
                        for qc in range(QC):
                            q0 = qc * QCHUNK
                            pc = psc.tile([NV, QCHUNK], f32, tag="pc", name=f"pc{hl}_{qc}")
                            for kb in range(KB):
                                ps = pss.tile(
                                    [128, QCHUNK], f32, tag="ps", name=f"sc{hl}_{qc}_{kb}"
                                )
                                at = attnp.tile(
                                    [128, QCHUNK], f16, tag="at", name=f"at{hl}_{qc}_{kb}"
                                )
                                for j in range(QCHUNK // 512):
                                    nc.tensor.matmul(
                                        ps[:, j * 512 : (j + 1) * 512],
                                        kt_sb[ob][r0 : r0 + 64, kb * 128 : (kb + 1) * 128],
                                        qt_sb[ob][r0 : r0 + 64, q0 + j * 512 : q0 + (j + 1) * 512],
                                        start=True,
                                        stop=True,
                                    )
                                # attn = exp(scores / sqrt(D)), cast to bf16
                                nc.scalar.activation(at[:], ps[:], Exp, scale=0.125)
                                for j in range(QCHUNK // 512):
                                    nc.tensor.matmul(
                                        pc[:, j * 512 : (j + 1) * 512],
                                        v_sb[kb][:, hl * NV : (hl + 1) * NV],
                                        at[:, j * 512 : (j + 1) * 512],
                                        start=(kb == 0),
                                        stop=(kb == KB - 1),
                                    )
                            nc.vector.tensor_copy(
                                cu_sb[ob][r0 : r0 + 64, q0 : q0 + QCHUNK], pc[0:D, :]
                            )
                            nc.vector.tensor_copy(
                                dent[hl][:, q0 : q0 + QCHUNK], pc[D : D + 1, :]
                            )
                    for h in range(HL if "B" in phases else 0):
                        nc.sync.dma_start(den8[h : h + 1, :], dent[h][:])

                # ============ Phase C: normalize ctx^T ============
                ctxt_sb = [
                    qkvp.tile([128, S], f16, tag=f"kt{ob}", name=f"ctxt{ob}")
                    for ob in range(OB)
                ]
                with (
                    tc.tile_pool(name="norm", bufs=1) as normp,
                    tc.tile_pool(name="psum_b", bufs=2, space="PSUM") as psb,
                ):
                    # 1/den via exp(-ln(den)) on ScalarE (ACT Reciprocal is banned)
                    logd = normp.tile([HL, S], f32, tag="logd")
                    recip = normp.tile([HL, S], f16, tag="recip")
                    if "C" in phases:
                        nc.scalar.activation(logd[:], den8[:], Ln)
                        nc.scalar.activation(recip[:], logd[:], Exp, scale=-1.0)
                    for ob in range(OB if "C" in phases else 0):
                        pb = psb.tile([128, S], f32, tag="pb", name=f"pb{ob}")
                        for j in range(S // 512):
                            nc.tensor.matmul(
                                pb[:, j * 512 : (j + 1) * 512],
                                sel_sb[:, ob, :],
                                recip[:, j * 512 : (j + 1) * 512],
                                start=True,
                                stop=True,
                            )
                        nc.vector.tensor_mul(ctxt_sb[ob][:], cu_sb[ob][:], pb[:])

            # ================= Phase D: output projection =================
            with (
                tc.tile_pool(name="outs", bufs=2) as outsp,
                tc.tile_pool(name="psum_o", bufs=2, space="PSUM") as pso,
            ):
                for eb in range(E // 128 if "D" in phases else 0):
                    po = pso.tile([128, S], f32, tag="po", name=f"po{eb}")
                    for oc in range(OB):
                        for j in range(S // 512):
                            nc.tensor.matmul(
                                po[:, j * 512 : (j + 1) * 512],
                                wo_sb[:, oc, eb * 128 : (eb + 1) * 128],
                                ctxt_sb[oc][:, j * 512 : (j + 1) * 512],
                                start=(oc == 0),
                                stop=(oc == OB - 1),
                            )
                    so = outsp.tile([128, S], f16, tag="so", name=f"so{eb}")
                    nc.scalar.copy(so[:], po[:])
                    [nc.sync, nc.scalar, nc.gpsimd][eb % 3].dma_start(
                        OUT[eb * 128 : (eb + 1) * 128, :], so[:]
                    )

    nc.compile()
    return nc


def _get_nc():
    if "nc" not in _CACHE:
        _CACHE["nc"] = _build()
    return _CACHE["nc"]


def _shard_inputs(Q, K, V, Wq, Wk, Wv, Wo):
    f16 = np.float16
    Q = np.asarray(Q, np.float32)
    K = np.asarray(K, np.float32)
    V = np.asarray(V, np.float32)
    Wq = np.asarray(Wq, np.float32)
    Wk = np.asarray(Wk, np.float32)
    Wv = np.asarray(Wv, np.float32)
    Wo = np.asarray(Wo, np.float32)

    sel = np.zeros((HL, OB, 128), np.float32)
    for ob in range(OB):
        sel[2 * ob, ob, 0:64] = 1.0
        sel[2 * ob + 1, ob, 64:128] = 1.0
    sel = sel.astype(f16)

    in_maps = []
    for c in range(N_CORES):
        b, g = divmod(c, 2)
        sl = slice(g * O, (g + 1) * O)
        in_maps.append(
            {
                "SEL": sel,
                "QT": np.ascontiguousarray(Q[b].T).astype(f16),
                "KT": np.ascontiguousarray(K[b].T).astype(f16),
                "VT": np.ascontiguousarray(V[b].T).astype(f16),
                "WQT": np.ascontiguousarray(Wq[sl, :].T).astype(f16),
                "WKT": np.ascontiguousarray(Wk[sl, :].T).astype(f16),
                "WVT": np.ascontiguousarray(Wv[sl, :].T).astype(f16),
                "WOT": np.ascontiguousarray(Wo[:, sl].T).astype(f16),
            }
        )
    return in_maps


class _Runner:
    """Compile-once executor for the SPMD bass program on 8 cores.

    Mirrors concourse.bass2jax.run_bass_via_pjrt but hoists the jit out of
    the call so repeated invocations don't re-trace/re-lower. With
    donate=False the output-shaped operands are not consumed, so calls can be
    chained (feeding outputs back in) to measure marginal device time.
    """

    def __init__(self, nc, donate=True):
        import jax
        import concourse.mybir as mybir
        from concourse import bass2jax

        bass2jax.install_neuronx_cc_hook()
        self.jax = jax
        self.nc = nc
        partition_name = (
            nc.partition_id_tensor.name if nc.partition_id_tensor else None
        )
        in_names, out_names, out_avals = [], [], []
        for alloc in nc.m.functions[0].allocations:
            if not isinstance(alloc, mybir.MemoryLocationSet):
                continue
            name = alloc.memorylocations[0].name
            if alloc.kind == "ExternalInput":
                if name != partition_name:
                    in_names.append(name)
            elif alloc.kind == "ExternalOutput":
                out_names.append(name)
                out_avals.append(
                    jax.core.ShapedArray(
                        tuple(alloc.tensor_shape), mybir.dt.np(alloc.dtype)
                    )
                )
        self.in_names = in_names
        self.out_names = out_names
        self.out_avals = out_avals
        n_params = len(in_names)
        n_outs = len(out_names)
        all_in_names = list(in_names) + list(out_names)
        if partition_name is not None:
            all_in_names.append(partition_name)
        all_in_names = tuple(all_in_names)

        def _body(*args):
            operands = list(args)
            if partition_name is not None:
                operands.append(bass2jax.partition_id_tensor())
            outs = bass2jax._bass_exec_p.bind(
                *operands,
                out_avals=tuple(out_avals),
                in_names=all_in_names,
                out_names=tuple(out_names),
                lowering_input_output_aliases=(),
                sim_require_finite=True,
                sim_require_nnan=True,
                nc=nc,
            )
            return tuple(outs)

        from concourse.bass2jax import Mesh, PartitionSpec, shard_map

        devices = jax.devices()[:N_CORES]
        mesh = Mesh(np.asarray(devices), ("core",))
        self.mesh = mesh
        self.pspec = PartitionSpec("core")
        self.sharded = jax.jit(
            shard_map(
                _body,
                mesh=mesh,
                in_specs=(PartitionSpec("core"),) * (n_params + n_outs),
                out_specs=(PartitionSpec("core"),) * n_outs,
                check_rep=False,
            ),
            donate_argnums=(
                tuple(range(n_params, n_params + n_outs)) if donate else ()
            ),
            keep_unused=True,
        )

    def concat_inputs(self, in_maps):
        return [
            np.concatenate([np.asarray(m[name]) for m in in_maps], axis=0)
            for name in self.in_names
        ]

    def zero_outs(self):
        return [
            np.zeros((N_CORES * a.shape[0], *a.shape[1:]), a.dtype)
            for a in self.out_avals
        ]

    def __call__(self, concat_in, concat_zeros=None):
        if concat_zeros is None:
            concat_zeros = self.zero_outs()
        out_arrs = self.sharded(*concat_in, *concat_zeros)
        return [
            {
                name: np.asarray(out_arrs[i]).reshape(
                    N_CORES, *self.out_avals[i].shape
                )[c]
                for i, name in enumerate(self.out_names)
            }
            for c in range(N_CORES)
        ]


def _get_runner():
    if "runner" not in _CACHE:
        _CACHE["runner"] = _Runner(_get_nc())
    return _CACHE["runner"]


def kernel(Q, K, V, mask, Wq, Wk, Wv, Wo):
    runner = _get_runner()
    in_maps = _shard_inputs(Q, K, V, Wq, Wk, Wv, Wo)
    results = runner(runner.concat_inputs(in_maps))
    out = np.empty((B, S, E), np.float32)
    for b in range(B):
        acc = results[2 * b]["OUT"].astype(np.float32) + results[2 * b + 1][
            "OUT"
        ].astype(np.float32)
        out[b] = acc.T
    return out


# revision 36
# speedup vs baseline: 12086.5289x; 1.9866x over previous
"""Trainium2 Bass kernel for 16-head MHA (B=4, S=2048, E=1024), 8 NeuronCores.

Sharding: core c handles batch b = c//2 and head-group g = c%2 (8 heads each).
Tensor-parallel within the head group: column-parallel Wq/Wk/Wv, row-parallel
Wo; the two partial Wo outputs per batch are summed on the host.

All device matmuls run in fp16 with fp32 PSUM accumulation. Inputs are
pre-transposed on the host to feature-major layouts so every matmul contracts
over the partition dimension with no on-device transposes:
  QT/KT/VT  [E, S]   (feature, token)
  WqT/WkT/WvT [E, O] (in-feature, local out-feature), O = 512
  WoT       [O, E]   (local out-feature, out)
Output per core: OUT [E, S] fp32 = partial (Wo @ ctx^T) for this head group.
"""

import sys

sys.path.insert(0, "/opt/trn_rl_repo")

import numpy as np

# Problem constants (hardcoded; kernel.py must be self-contained).
B = 4
S = 2048
E = 1024
H = 16
D = 64
N_CORES = 8
HL = H // 2  # heads per core (head-group of 8)
O = HL * D  # 512 local output features of the q/k/v projections
IC = E // 128  # 8 contraction chunks for projections
OB = O // 128  # 4 output row-blocks (head pairs)
TB = S // 128  # 16 token blocks
KB = S // 128  # 16 key blocks per head
QCHUNK = 1024  # q columns processed per softmax tile
QC = S // QCHUNK  # 2
NV = 65  # v columns + 1 ones column for the softmax denominator

_CACHE = {}


def _build(phases="ABCD"):
    import concourse.bass as bass
    import concourse.mybir as mybir
    from concourse import bacc, tile

    f32 = mybir.dt.float32
    f16 = mybir.dt.float16
    Exp = mybir.ActivationFunctionType.Exp
    Ln = mybir.ActivationFunctionType.Ln

    nc = bacc.Bacc(None, target_bir_lowering=False)

    # Packed inputs: one giant DMA per DGE ring (SP/ACT/SWDGE). Each of
    # INQ/INK/INV is, per contraction chunk ic, the activation chunk [128, S]
    # followed by the matching projection-weight chunk [128, O], laid out
    # partition-major. INW is the packed WoT chunks.
    XW = S + O
    INQ = nc.dram_tensor("INQ", [128, IC * XW], f16, kind="ExternalInput")
    INK = nc.dram_tensor("INK", [128, IC * XW], f16, kind="ExternalInput")
    INV = nc.dram_tensor("INV", [128, IC * XW], f16, kind="ExternalInput")
    INW = nc.dram_tensor("INW", [128, OB * E], f16, kind="ExternalInput")
    # Selector matrices for broadcasting softmax reciprocals (host-built):
    # SEL[ob].T @ recip -> [128, S] whose rows 0-63 replicate recip[2*ob]
    # and rows 64-127 replicate recip[2*ob+1].
    SEL = nc.dram_tensor("SEL", [HL, OB, 128], f16, kind="ExternalInput")
    OUT = nc.dram_tensor("OUT", [E, S], f16, kind="ExternalOutput")

    with tile.TileContext(nc) as tc:
        with (
            tc.tile_pool(name="consts", bufs=1) as constp,
            tc.tile_pool(name="weights", bufs=1) as wp,
            tc.tile_pool(name="qkv", bufs=1) as qkvp,
        ):
            sel_sb = constp.tile([HL, OB, 128], f16, tag="sel")
            nc.scalar.dma_start(sel_sb[:], SEL[:])

            wo_sb = wp.tile([128, OB, E], f16, tag="wo")
            nc.sync.dma_start(wo_sb[:].rearrange("p a b -> p (a b)"), INW[:])

            # ---- persistent qT/kT/v in SBUF (f16) ----
            # qT/kT tiles: [128 rows = 2 heads x 64 dims, S]
            qt_sb = [qkvp.tile([128, S], f16, tag=f"qt{ob}", name=f"qt{ob}") for ob in range(OB)]
            kt_sb = [qkvp.tile([128, S], f16, tag=f"kt{ob}", name=f"kt{ob}") for ob in range(OB)]
            # v tiles: [128 tokens, 8 heads x (64 v-dims + ones col)]
            v_sb = [qkvp.tile([128, HL * NV], f16, tag=f"v{tb}", name=f"v{tb}") for tb in range(TB)]
            for tb in range(TB):
                ones_col = v_sb[tb].rearrange("p (h x) -> p h x", x=NV)[:, :, D : D + 1]
                nc.vector.memset(ones_col, 1.0)

            # ================= Phase A: projections =================
            with (
                tc.tile_pool(name="pack", bufs=1) as packp,
                tc.tile_pool(name="psum_a", bufs=8, space="PSUM") as psa,
            ):
                # one giant packed DMA per ring; slices of these tiles serve
                # as both activations and projection weights below
                inq = packp.tile([128, IC, XW], f16, tag="inq")
                ink = packp.tile([128, IC, XW], f16, tag="ink")
                inv = packp.tile([128, IC, XW], f16, tag="inv")
                nc.sync.dma_start(inq[:].rearrange("p a b -> p (a b)"), INQ[:])
                nc.scalar.dma_start(ink[:].rearrange("p a b -> p (a b)"), INK[:])
                nc.gpsimd.dma_start(inv[:].rearrange("p a b -> p (a b)"), INV[:])

                # q and k projections: out rows = local feature block, cols = tokens
                for which, pk, dst in (("q", inq, qt_sb), ("k", ink, kt_sb)):
                    for ob in range(OB if "Y" not in phases else 0):
                        # ic outer so each stationary weight block is loaded
                        # once and reused for all four moving chunks
                        pss4 = [
                            psa.tile([128, 512], f32, tag="ps", name=f"ps_{which}{ob}_{j}")
                            for j in range(S // 512)
                        ]
                        for ic in range(IC):
                            for j in range(S // 512):
                                nc.tensor.matmul(
                                    pss4[j][:],
                                    pk[:, ic, S + ob * 128 : S + (ob + 1) * 128],
                                    pk[:, ic, j * 512 : (j + 1) * 512],
                                    start=(ic == 0),
                                    stop=(ic == IC - 1),
                                )
                        for j in range(S // 512):
                            nc.vector.tensor_copy(
                                dst[ob][:, j * 512 : (j + 1) * 512], pss4[j][:]
                            )

                # v projection: out rows = tokens, cols = local features
                for tb in range(TB if "Y" not in phases else 0):
                    ps = psa.tile([128, 512], f32, tag="ps", name=f"ps_v{tb}")
                    for ic in range(IC):
                        nc.tensor.matmul(
                            ps[:],
                            inv[:, ic, tb * 128 : (tb + 1) * 128],
                            inv[:, ic, S : S + O],
                            start=(ic == 0),
                            stop=(ic == IC - 1),
                        )
                    vdst = v_sb[tb].rearrange("p (h x) -> p h x", x=NV)[:, :, 0:D]
                    nc.scalar.copy(vdst, ps[:].rearrange("p (h d) -> p h d", d=D))

            # ================= Phase B: attention =================
            with tc.tile_pool(name="cun", bufs=1) as cunp:
                # unnormalized ctx^T (fp32): [128 rows = 2 heads x 64 dims, S]
                cu_sb = [cunp.tile([128, S], f32, tag=f"cu{ob}", name=f"cu{ob}") for ob in range(OB)]
                # softmax denominators: per-head [1, S] staging rows (compute
                # engines can only address base partitions 0/32/64, so rows are
                # gathered into den8 with SBUF->SBUF DMAs afterwards)
                dent = [cunp.tile([1, S], f32, tag=f"dent{h}", name=f"dent{h}") for h in range(HL)]
                den8 = cunp.tile([HL, S], f32, tag="den8")

                with (
                    tc.tile_pool(name="attn", bufs=3) as attnp,
                    tc.tile_pool(name="psum_s", bufs=2, space="PSUM") as pss,
                    tc.tile_pool(name="psum_c", bufs=2, space="PSUM") as psc,
                ):
                    for hl in range(HL if "B" in phases else 0):
                        ob = hl // 2
                        r0 = (hl % 2) * 64
                        for qc in range(QC):
                            q0 = qc * QCHUNK
                            pc = psc.tile([NV, QCHUNK], f32, tag="pc", name=f"pc{hl}_{qc}")
                            for kb in range(KB):
                                ps = pss.tile(
                                    [128, QCHUNK], f32, tag="ps", name=f"sc{hl}_{qc}_{kb}"
                                )
                                at = attnp.tile(
                                    [128, QCHUNK], f16, tag="at", name=f"at{hl}_{qc}_{kb}"
                                )
                                for j in range(QCHUNK // 512):
                                    nc.tensor.matmul(
                                        ps[:, j * 512 : (j + 1) * 512],
                                        kt_sb[ob][r0 : r0 + 64, kb * 128 : (kb + 1) * 128],
                                        qt_sb[ob][r0 : r0 + 64, q0 + j * 512 : q0 + (j + 1) * 512],
                                        start=True,
                                        stop=True,
                                    )
                                # attn = exp(scores / sqrt(D)), cast to bf16
                                nc.scalar.activation(at[:], ps[:], Exp, scale=0.125)
                                for j in range(QCHUNK // 512):
                                    nc.tensor.matmul(
                                        pc[:, j * 512 : (j + 1) * 512],
                                        v_sb[kb][:, hl * NV : (hl + 1) * NV],
                                        at[:, j * 512 : (j + 1) * 512],
                                        start=(kb == 0),
                                        stop=(kb == KB - 1),
                                    )
                            nc.vector.tensor_copy(
                                cu_sb[ob][r0 : r0 + 64, q0 : q0 + QCHUNK], pc[0:D, :]
                            )
                            nc.vector.tensor_copy(
                                dent[hl][:, q0 : q0 + QCHUNK], pc[D : D + 1, :]
                            )
                    for h in range(HL if "B" in phases else 0):
                        nc.sync.dma_start(den8[h : h + 1, :], dent[h][:])

                # ============ Phase C: normalize ctx^T ============
                ctxt_sb = [
                    qkvp.tile([128, S], f16, tag=f"kt{ob}", name=f"ctxt{ob}")
                    for ob in range(OB)
                ]
                with (
                    tc.tile_pool(name="norm", bufs=1) as normp,
                    tc.tile_pool(name="psum_b", bufs=2, space="PSUM") as psb,
                ):
                    # 1/den via exp(-ln(den)) on ScalarE (ACT Reciprocal is banned)
                    logd = normp.tile([HL, S], f32, tag="logd")
                    recip = normp.tile([HL, S], f16, tag="recip")
                    if "C" in phases:
                        nc.scalar.activation(logd[:], den8[:], Ln)
                        nc.scalar.activation(recip[:], logd[:], Exp, scale=-1.0)
                    for ob in range(OB if "C" in phases else 0):
                        pb = psb.tile([128, S], f32, tag="pb", name=f"pb{ob}")
                        for j in range(S // 512):
                            nc.tensor.matmul(
                                pb[:, j * 512 : (j + 1) * 512],
                                sel_sb[:, ob, :],
                                recip[:, j * 512 : (j + 1) * 512],
                                start=True,
                                stop=True,
                            )
                        nc.vector.tensor_mul(ctxt_sb[ob][:], cu_sb[ob][:], pb[:])

            # ================= Phase D: output projection =================
            with (
                tc.tile_pool(name="outs", bufs=2) as outsp,
                tc.tile_pool(name="psum_o", bufs=2, space="PSUM") as pso,
            ):
                for eb in range(E // 128 if "D" in phases else 0):
                    po = pso.tile([128, S], f32, tag="po", name=f"po{eb}")
                    for oc in range(OB):
                        for j in range(S // 512):
                            nc.tensor.matmul(
                                po[:, j * 512 : (j + 1) * 512],
                                wo_sb[:, oc, eb * 128 : (eb + 1) * 128],
                                ctxt_sb[oc][:, j * 512 : (j + 1) * 512],
                                start=(oc == 0),
                                stop=(oc == OB - 1),
                            )
                    so = outsp.tile([128, S], f16, tag="so", name=f"so{eb}")
                    nc.scalar.copy(so[:], po[:])
                    [nc.sync, nc.scalar, nc.gpsimd][eb % 3].dma_start(
                        OUT[eb * 128 : (eb + 1) * 128, :], so[:]
                    )

    nc.compile()
    return nc


def _get_nc():
    if "nc" not in _CACHE:
        _CACHE["nc"] = _build()
    return _CACHE["nc"]


def _shard_inputs(Q, K, V, Wq, Wk, Wv, Wo):
    f16 = np.float16
    Q = np.asarray(Q, np.float32)
    K = np.asarray(K, np.float32)
    V = np.asarray(V, np.float32)
    Wq = np.asarray(Wq, np.float32)
    Wk = np.asarray(Wk, np.float32)
    Wv = np.asarray(Wv, np.float32)
    Wo = np.asarray(Wo, np.float32)

    sel = np.zeros((HL, OB, 128), np.float32)
    for ob in range(OB):
        sel[2 * ob, ob, 0:64] = 1.0
        sel[2 * ob + 1, ob, 64:128] = 1.0
    sel = sel.astype(f16)

    def pack_xw(XTb, WTg):
        # [128, IC*(S+O)]: per chunk ic, activation chunk then weight chunk
        parts = []
        for ic in range(IC):
            parts.append(XTb[ic * 128 : (ic + 1) * 128, :])
            parts.append(WTg[ic * 128 : (ic + 1) * 128, :])
        return np.ascontiguousarray(np.concatenate(parts, axis=1)).astype(f16)

    in_maps = []
    for c in range(N_CORES):
        b, g = divmod(c, 2)
        sl = slice(g * O, (g + 1) * O)
        wot = Wo[:, sl].T  # [O, E]
        inw = np.concatenate(
            [wot[oc * 128 : (oc + 1) * 128, :] for oc in range(OB)], axis=1
        )
        in_maps.append(
            {
                "SEL": sel,
                "INQ": pack_xw(Q[b].T, Wq[sl, :].T),
                "INK": pack_xw(K[b].T, Wk[sl, :].T),
                "INV": pack_xw(V[b].T, Wv[sl, :].T),
                "INW": np.ascontiguousarray(inw).astype(f16),
            }
        )
    return in_maps


class _Runner:
    """Compile-once executor for the SPMD bass program on 8 cores.

    Mirrors concourse.bass2jax.run_bass_via_pjrt but hoists the jit out of
    the call so repeated invocations don't re-trace/re-lower. With
    donate=False the output-shaped operands are not consumed, so calls can be
    chained (feeding outputs back in) to measure marginal device time.
    """

    def __init__(self, nc, donate=True):
        import jax
        import concourse.mybir as mybir
        from concourse import bass2jax

        bass2jax.install_neuronx_cc_hook()
        self.jax = jax
        self.nc = nc
        partition_name = (
            nc.partition_id_tensor.name if nc.partition_id_tensor else None
        )
        in_names, out_names, out_avals = [], [], []
        for alloc in nc.m.functions[0].allocations:
            if not isinstance(alloc, mybir.MemoryLocationSet):
                continue
            name = alloc.memorylocations[0].name
            if alloc.kind == "ExternalInput":
                if name != partition_name:
                    in_names.append(name)
            elif alloc.kind == "ExternalOutput":
                out_names.append(name)
                out_avals.append(
                    jax.core.ShapedArray(
                        tuple(alloc.tensor_shape), mybir.dt.np(alloc.dtype)
                    )
                )
        self.in_names = in_names
        self.out_names = out_names
        self.out_avals = out_avals
        n_params = len(in_names)
        n_outs = len(out_names)
        all_in_names = list(in_names) + list(out_names)
        if partition_name is not None:
            all_in_names.append(partition_name)
        all_in_names = tuple(all_in_names)

        def _body(*args):
            operands = list(args)
            if partition_name is not None:
                operands.append(bass2jax.partition_id_tensor())
            outs = bass2jax._bass_exec_p.bind(
                *operands,
                out_avals=tuple(out_avals),
                in_names=all_in_names,
                out_names=tuple(out_names),
                lowering_input_output_aliases=(),
                sim_require_finite=True,
                sim_require_nnan=True,
                nc=nc,
            )
            return tuple(outs)

        from concourse.bass2jax import Mesh, PartitionSpec, shard_map

        devices = jax.devices()[:N_CORES]
        mesh = Mesh(np.asarray(devices), ("core",))
        self.mesh = mesh
        self.pspec = PartitionSpec("core")
        self.sharded = jax.jit(
            shard_map(
                _body,
                mesh=mesh,
                in_specs=(PartitionSpec("core"),) * (n_params + n_outs),
                out_specs=(PartitionSpec("core"),) * n_outs,
                check_rep=False,
            ),
            donate_argnums=(
                tuple(range(n_params, n_params + n_outs)) if donate else ()
            ),
            keep_unused=True,
        )

    def concat_inputs(self, in_maps):
        return [
            np.concatenate([np.asarray(m[name]) for m in in_maps], axis=0)
            for name in self.in_names
        ]

    def zero_outs(self):
        return [
            np.zeros((N_CORES * a.shape[0], *a.shape[1:]), a.dtype)
            for a in self.out_avals
        ]

    def __call__(self, concat_in, concat_zeros=None):
        if concat_zeros is None:
            concat_zeros = self.zero_outs()
        out_arrs = self.sharded(*concat_in, *concat_zeros)
        return [
            {
                name: np.asarray(out_arrs[i]).reshape(
                    N_CORES, *self.out_avals[i].shape
                )[c]
                for i, name in enumerate(self.out_names)
            }
            for c in range(N_CORES)
        ]


def _get_runner():
    if "runner" not in _CACHE:
        _CACHE["runner"] = _Runner(_get_nc())
    return _CACHE["runner"]


def kernel(Q, K, V, mask, Wq, Wk, Wv, Wo):
    runner = _get_runner()
    in_maps = _shard_inputs(Q, K, V, Wq, Wk, Wv, Wo)
    results = runner(runner.concat_inputs(in_maps))
    out = np.empty((B, S, E), np.float32)
    for b in range(B):
        acc = results[2 * b]["OUT"].astype(np.float32) + results[2 * b + 1][
            "OUT"
        ].astype(np.float32)
        out[b] = acc.T
    return out
